# revision 21
# baseline (speedup 1.0000x reference)
"""Trainium2 Bass kernel for a fused multi-head attention block.

Reference computation (B=4, T=2048, D=1152, H=8, HD=144, full rotary):
    q,k,v = x@Wq.T, x@Wk.T, x@Wv.T   (per head)
    q,k   = rope(q, k, cos, sin)
    o     = softmax(q k^T / sqrt(HD)) v
    out   = o @ Wo.T

Sharding (8 cores): core c = (batch b = c//2, head-group hg = c%2).
Each core computes 4 heads of one batch and a partial output
out_part = o_local @ Wo[:, hg_cols].T ; host sums the two partials per batch.

Per-core structure (v2 — rebuilt from trace analysis of v1, 476us -> 391us):
  * Projections: q and k are computed in ONE merged pass (wqkT [D,1152] =
    [q a-blocks 512 | k a-blocks 512 | q b-dims 64 | k b-dims 64]) with 3
    matmuls per (n,k-chunk) sharing the x stationary (LDW amortized 1:3).
  * rope reads the projection PSUM directly (no copy): 4 DVE muls using a
    host-precomputed sign-folded/permuted sin table (snP) make every
    combine a plain DVE add; combines write bf16 q/k tiles which PE
    transposes to [e,t].  Transpose results copy out on the Scalar engine
    (idle during phase A).
  * Inputs arrive as ONE DMA descriptor per logical transfer (the Sync
    engine serializes descriptors at ~630 ns — v1's ~90 descriptors gated
    the v projection).  x is piece-major (all 9 D-chunks of each 256-col
    t-piece together) and the first wave is chunk-granular so matmul
    (n=0,k) unblocks as pair k lands.  Dummy warmup matmuls keep HAM at
    K=8/8 through the initial DMA window.  The score-tail b-block replicas
    are DMAed in two column halves so the first half issues mid-phase-A.
  * Attention processes TWO 512-wide q-blocks at once per (head, kt):
    scores S^T [keys, 1024q] into a 2-bank PSUM tile.  Per key-tile: two
    K=32 b-block tails FIRST (start=True, distinct tile_position
    row-groups -> concurrent in PE), then two K=128 mains sharing the kTa
    stationary (LDWEIGHTS hidden), ONE [128,1024] Exp (halves the ACT
    fixed overhead that paced v1), then 8 PV matmuls into 3 PSUM banks of
    packed 145-wide accumulators (softmax denominator via ones column of
    v).  The kt loop is software-pipelined depth 2 (scores run two
    key-tiles ahead of PV) so the PE never idles on scores->exp->PV
    latency; O1 is double-buffered so the next head's PV doesn't wait on
    the normalize.  PSUM budget: 2x2-bank sps + 2+1+1 accumulator banks
    = 8 (the binding constraint throughout).
  * Final projection in bf16: o normalized straight to bf16, PE transpose,
    then k-outer/j3-inner matmuls sharing the o^T stationary 1:3.
  * dtypes: all matmuls bf16 (f32 accum in PSUM); output f32.
"""

import numpy as np

B, T, D, H = 4, 2048, 1152, 8
HL = 4              # heads per core
HD = 144            # head dim
DV = HL * HD        # 576, v/o width
WQK = 1152          # merged q/k projection width: 512 + 512 + 64 + 64
NT = T // 128       # 16 t-tiles
KC = D // 128       # 9 contraction chunks
SCALE = float(HD) ** -0.5
NCORES = 8

_NC_CACHE = {}


def _build(debug=False):
    import concourse.bacc as bacc
    import concourse.mybir as mybir
    from concourse.tile import TileContext

    dt = mybir.dt
    f32, bf16 = dt.float32, dt.bfloat16
    AF = mybir.ActivationFunctionType

    nc = bacc.Bacc(
        "TRN2",
        target_bir_lowering=False,
        debug=debug,
        enable_asserts=False,
        num_devices=NCORES,
    )

    xT = nc.declare_dram_parameter("xT", [D, T], bf16, isOutput=False)
    wqkT = nc.declare_dram_parameter("wqkT", [D, WQK], bf16, isOutput=False)
    wvT = nc.declare_dram_parameter("wvT", [D, DV], bf16, isOutput=False)
    woT = nc.declare_dram_parameter("woT", [DV, D], bf16, isOutput=False)
    cosN = nc.declare_dram_parameter("cosN", [T, HD], bf16, isOutput=False)
    snPN = nc.declare_dram_parameter("snPN", [T, HD], bf16, isOutput=False)
    identB = nc.declare_dram_parameter("identB", [128, 128], bf16, isOutput=False)
    out = nc.declare_dram_parameter("out", [T, D], f32, isOutput=True)

    with TileContext(nc) as tc:
        with tc.tile_pool(name="persist", bufs=1) as P0:
            ident_bf = P0.tile([128, 128], bf16, name="ident_bf", tag="ident_bf")
            nc.sync.dma_start(ident_bf[:], identB[:])

            qTa = [
                P0.tile([128, T], bf16, name=f"qTa{h}", tag=f"qTa{h}")
                for h in range(HL)
            ]
            kTa = [
                P0.tile([128, T], bf16, name=f"kTa{h}", tag=f"kTa{h}")
                for h in range(HL)
            ]
            qTB = P0.tile([128, T], bf16, name="qTB", tag="qTB")
            kTB = P0.tile([128, T], bf16, name="kTB", tag="kTB")
            # per-head replicas of the b-block rows at all four 32-row groups,
            # so four score-tail K=32 matmuls can issue to distinct PE
            # row-groups and overlap in the array
            qTBr = [
                P0.tile([128, T], bf16, name=f"qTBr{h}", tag=f"qTBr{h}")
                for h in range(HL)
            ]
            kTBr = [
                P0.tile([128, T], bf16, name=f"kTBr{h}", tag=f"kTBr{h}")
                for h in range(HL)
            ]
            vt = [
                P0.tile([128, HL * (HD + 1)], bf16, name=f"v{t}", tag=f"v{t}")
                for t in range(NT)
            ]

            # ---------------- Phase A: projections + rope + transposes -----
            with tc.tile_pool(name="pa", bufs=1) as pa:

                # single SBUF tiles with one DMA descriptor per logical
                # transfer (3D access patterns) — v1/v2 used ~90 descriptors
                # which serialized on the Sync engine (~630 ns each) and
                # gated the first half of the v projection
                xbig = pa.tile([128, KC * T], bf16, name="xbig", tag="xbig")
                x3 = xbig.rearrange("p (c t) -> p c t", c=KC)
                xs = xT.rearrange("(c p) t -> p c t", p=128)
                xtiles = [x3[:, k] for k in range(KC)]
                cos_sb = pa.tile([128, NT * HD], bf16, name="cos_sb", tag="cos_sb")
                snp_sb = pa.tile([128, NT * HD], bf16, name="snp_sb", tag="snp_sb")
                wvbig = pa.tile([128, KC * DV], bf16, name="wvbig", tag="wvbig")
                wv3 = wvbig.rearrange("p (c e) -> p c e", c=KC)
                wv_tiles = [wv3[:, k] for k in range(KC)]
                # first wave at chunk granularity (wv_k + x[k, piece0]
                # interleaved) so matmul (n=0,k) unblocks as pair k lands
                # instead of after the whole wv + piece transfers; later x
                # pieces are single descriptors (Sync-engine issue is
                # ~630 ns per descriptor)
                NP = 8
                PW = T // NP
                wvs = wvT.rearrange("(c p) e -> p c e", p=128)
                for k in range(KC):
                    nc.sync.dma_start(wv3[:, k], wvs[:, k])
                    nc.sync.dma_start(
                        x3[:, k, 0:PW], xs[:, k, 0:PW]
                    )
                for p in range(1, NP):
                    nc.sync.dma_start(
                        x3[:, :, p * PW : (p + 1) * PW],
                        xs[:, :, p * PW : (p + 1) * PW],
                    )

                # ---- v projection ----
                with tc.tile_pool(name="pavps", bufs=1, space="PSUM") as pavps:
                    # dummy matmuls bridge the initial DMA window so HAM
                    # reaches K=8/8 before the first real matmul
                    warm = pavps.tile(
                        [128, 128], f32, name="warm", tag="warm", bufs=1
                    )
                    with tc.high_priority():
                        for _ in range(10):
                            nc.tensor.matmul(
                                warm[:], ident_bf[:], ident_bf[:],
                                start=True, stop=True,
                            )
                    for n in range(NT):
                        ps_v = pavps.tile(
                            [128, DV], f32, name="ps_v", tag="pv", bufs=2
                        )
                        for k in range(KC):
                            st, sp = k == 0, k == KC - 1
                            lhs = xtiles[k][:, n * 128 : (n + 1) * 128]
                            nc.tensor.matmul(
                                ps_v[:, 0:512], lhs, wv_tiles[k][:, 0:512],
                                start=st, stop=sp,
                            )
                            nc.tensor.matmul(
                                ps_v[:, 512:DV], lhs, wv_tiles[k][:, 512:DV],
                                start=st, stop=sp,
                            )
                        v3 = vt[n].rearrange("p (h e) -> p h e", h=HL)
                        nc.vector.tensor_copy(
                            v3[:, :, 0:HD],
                            ps_v.rearrange("p (h e) -> p h e", h=HL),
                        )
                        nc.gpsimd.memset(v3[:, :, HD : HD + 1], 1.0)

                # ---- merged q/k projection weights + trig tables ----
                wqkbig = pa.tile([128, KC * WQK], bf16, name="wqkbig", tag="wqkbig")
                wqk3 = wqkbig.rearrange("p (c e) -> p c e", c=KC)
                wqk_tiles = [wqk3[:, k] for k in range(KC)]
                nc.sync.dma_start(
                    wqk3, wqkT.rearrange("(c p) e -> p c e", p=128)
                )
                nc.sync.dma_start(
                    cos_sb.rearrange("p (n r) -> p n r", n=NT),
                    cosN.rearrange("(n p) r -> p n r", p=128),
                )
                nc.sync.dma_start(
                    snp_sb.rearrange("p (n r) -> p n r", n=NT),
                    snPN.rearrange("(n p) r -> p n r", p=128),
                )

                def trig3(sb, n):
                    # [128, 144] row block for t-tile n, broadcast over 4 heads
                    return (
                        sb[:, n * HD : (n + 1) * HD]
                        .rearrange("p (o r) -> p o r", o=1)
                        .to_broadcast([128, HL, HD])
                    )

                def rope_one(ps_a, ps_bq, qtl, cos3, snp3):
                    """ps_a [128,512] f32 (4 a-blocks), ps_bq [128,4,16] f32
                    view of the packed b dims -> qtl [128,640] bf16 with
                    rotary applied (layout: 4x128 a-blocks | 4x(16+16pad)).

                    m1[j] = q[j]*cos[j]; m2[j] = q[j]*snP[j] where
                    snP[i] = sin[(i+72)%144] * (+1 if i<72 else -1), so every
                    combine is a plain add: out[j] = m1[j] + m2[(j+72)%144].
                    """
                    pa3 = ps_a.rearrange("p (h e) -> p h e", h=HL)
                    m1 = pa.tile([128, 576], bf16, name="m1", tag="m1", bufs=2)
                    m2 = pa.tile([128, 576], bf16, name="m2", tag="m2", bufs=2)
                    m1a = m1[:, 0:512].rearrange("p (h e) -> p h e", h=HL)
                    m1b = m1[:, 512:576].rearrange("p (h e) -> p h e", h=HL)
                    m2a = m2[:, 0:512].rearrange("p (h e) -> p h e", h=HL)
                    m2b = m2[:, 512:576].rearrange("p (h e) -> p h e", h=HL)
                    v = nc.vector
                    v.tensor_mul(m1a[:], pa3[:], cos3[:, :, 0:128])
                    v.tensor_mul(m1b[:], ps_bq[:], cos3[:, :, 128:144])
                    v.tensor_mul(m2a[:], pa3[:], snp3[:, :, 0:128])
                    v.tensor_mul(m2b[:], ps_bq[:], snp3[:, :, 128:144])
                    oa = qtl[:, 0:512].rearrange("p (h e) -> p h e", h=HL)
                    ob = qtl[:, 512:640].rearrange("p (h e) -> p h e", h=HL)
                    # all-bf16 SBUF operands -> DVE 4x fast mode
                    v.tensor_add(oa[:, :, 0:56], m1a[:, :, 0:56], m2a[:, :, 72:128])
                    v.tensor_add(oa[:, :, 56:72], m1a[:, :, 56:72], m2b[:, :, 0:16])
                    v.tensor_add(oa[:, :, 72:128], m1a[:, :, 72:128], m2a[:, :, 0:56])
                    v.tensor_add(ob[:, :, 0:16], m1b[:], m2a[:, :, 56:72])
                    nc.gpsimd.memset(ob[:, :, 16:32], 0.0)

                with tc.tile_pool(name="paqps", bufs=1, space="PSUM") as paqps:

                    def transposes(n, qtl, ktl):
                        for src, dsts, dstb in ((qtl, qTa, qTB), (ktl, kTa, kTB)):
                            for j in range(5):
                                tp = paqps.tile(
                                    [128, 128], bf16, name="tp", tag="tp", bufs=3
                                )
                                nc.tensor.transpose(
                                    tp[:], src[:, 128 * j : 128 * (j + 1)], ident_bf[:]
                                )
                                dst = dsts[j] if j < 4 else dstb
                                nc.scalar.copy(
                                    dst[:, n * 128 : (n + 1) * 128], tp[:]
                                )

                    pend = None
                    for n in range(NT):
                        ps_q = paqps.tile(
                            [128, 512], f32, name="ps_q", tag="psq", bufs=2
                        )
                        ps_k = paqps.tile(
                            [128, 512], f32, name="ps_k", tag="psk", bufs=2
                        )
                        ps_b = paqps.tile(
                            [128, 128], f32, name="ps_b", tag="psb", bufs=1
                        )
                        for k in range(KC):
                            st, sp = k == 0, k == KC - 1
                            lhs = xtiles[k][:, n * 128 : (n + 1) * 128]
                            nc.tensor.matmul(
                                ps_q[:], lhs, wqk_tiles[k][:, 0:512],
                                start=st, stop=sp,
                            )
                            nc.tensor.matmul(
                                ps_k[:], lhs, wqk_tiles[k][:, 512:1024],
                                start=st, stop=sp,
                            )
                            nc.tensor.matmul(
                                ps_b[:], lhs, wqk_tiles[k][:, 1024:1152],
                                start=st, stop=sp,
                            )
                        if pend is not None:
                            transposes(*pend)
                        qtl = pa.tile([128, 640], bf16, name="qtl", tag="qtl", bufs=2)
                        ktl = pa.tile([128, 640], bf16, name="ktl", tag="ktl", bufs=2)
                        c3, s3 = trig3(cos_sb, n), trig3(snp_sb, n)
                        rope_one(
                            ps_q,
                            ps_b[:, 0:64].rearrange("p (h e) -> p h e", h=HL),
                            qtl, c3, s3,
                        )
                        rope_one(
                            ps_k,
                            ps_b[:, 64:128].rearrange("p (h e) -> p h e", h=HL),
                            ktl, c3, s3,
                        )
                        pend = (n, qtl, ktl)
                    transposes(*pend)

                # replicate the b-blocks in two column halves so the
                # first half's DMAs issue as soon as transposes n<=7 are
                # done (subtile deps) instead of after the whole phase;
                # h-major so head 0 lands first
                for half in range(2):
                    cl, cr = half * 1024, (half + 1) * 1024
                    for hh in range(HL):
                        for j in range(4):
                            nc.sync.dma_start(
                                qTBr[hh][32 * j : 32 * j + 32, cl:cr],
                                qTB[32 * hh : 32 * hh + 32, cl:cr],
                            )
                            nc.sync.dma_start(
                                kTBr[hh][32 * j : 32 * j + 32, cl:cr],
                                kTB[32 * hh : 32 * hh + 32, cl:cr],
                            )

            # ---------------- Phase B: attention --------------------------
            with tc.tile_pool(name="pb", bufs=1) as pb:
                ot = [
                    pb.tile([128, DV], bf16, name=f"o{t}", tag=f"o{t}")
                    for t in range(NT)
                ]
                with tc.tile_pool(name="pbps", bufs=1, space="PSUM") as pbps:
                    HD1 = HD + 1
                    for p in range(2):
                        qof = p * 1024
                        for h in range(HL):
                            # 8 PV accumulators packed into 3 PSUM banks;
                            # O1 double-buffered so the next (h,p)'s first PV
                            # matmuls don't wait on this one's normalize
                            O1 = pbps.tile(
                                [128, 3 * HD1], f32, name="O1", tag="O1", bufs=2
                            )
                            O2 = pbps.tile(
                                [128, 3 * HD1], f32, name="O2", tag="O2", bufs=1
                            )
                            O3 = pbps.tile(
                                [128, 2 * HD1], f32, name="O3", tag="O3", bufs=1
                            )
                            o_ps = (
                                [O1[:, i * HD1 : (i + 1) * HD1] for i in range(3)]
                                + [O2[:, i * HD1 : (i + 1) * HD1] for i in range(3)]
                                + [O3[:, i * HD1 : (i + 1) * HD1] for i in range(2)]
                            )

                            def scores_one(kt):
                                # one key-tile into one 2-bank PSUM tile:
                                # per 512-q half, a K=32 b-block tail first
                                # (start=True, two tails on distinct PE
                                # row-groups run concurrently) then the
                                # K=128 main carrying stop; the two mains
                                # share the kTa stationary so its
                                # LDWEIGHTS is hidden. ONE [128,1024] exp.
                                sps = pbps.tile(
                                    [128, 1024], f32, name="sps", tag="sc", bufs=2
                                )
                                for half in range(2):
                                    rg = 2 * (kt % 2) + half
                                    nc.tensor.matmul(
                                        sps[:, 512 * half : 512 * (half + 1)],
                                        kTBr[h][
                                            32 * rg : 32 * rg + 32,
                                            kt * 128 : (kt + 1) * 128,
                                        ],
                                        qTBr[h][
                                            32 * rg : 32 * rg + 32,
                                            qof + 512 * half : qof + 512 * (half + 1),
                                        ],
                                        start=True,
                                        stop=False,
                                        tile_position=(32 * rg, 0),
                                    )
                                for half in range(2):
                                    nc.tensor.matmul(
                                        sps[:, 512 * half : 512 * (half + 1)],
                                        kTa[h][:, kt * 128 : (kt + 1) * 128],
                                        qTa[h][
                                            :,
                                            qof + 512 * half : qof + 512 * (half + 1),
                                        ],
                                        start=False,
                                        stop=True,
                                    )
                                E = pb.tile(
                                    [128, 1024], bf16, name="E", tag="E", bufs=6
                                )
                                nc.scalar.activation(
                                    E[:], sps[:], AF.Exp, scale=SCALE
                                )
                                return E

                            def pv_one(kt, E):
                                for ql in range(8):
                                    # start/stop are carried by the first/
                                    # last matmul touching each bank
                                    st = kt == 0 and ql in (0, 3, 6)
                                    sp = kt == NT - 1 and ql in (2, 5, 7)
                                    nc.tensor.matmul(
                                        o_ps[ql][:],
                                        E[:, ql * 128 : (ql + 1) * 128],
                                        vt[kt][:, HD1 * h : HD1 * (h + 1)],
                                        start=st,
                                        stop=sp,
                                    )

                            # software pipeline depth 2: scores run two
                            # key-tiles ahead of PV, so when EXP(kt)
                            # completes the PE has both PV(kt) and
                            # scores(kt+2) ready and never idles on the
                            # scores->exp->PV latency chain
                            eq = [scores_one(0), scores_one(1)]
                            for kt in range(NT):
                                if kt + 2 < NT:
                                    eq.append(scores_one(kt + 2))
                                pv_one(kt, eq.pop(0))
                            last_hp = p == 1 and h == HL - 1
                            for ql in range(8):
                                t = 8 * p + ql
                                r = pb.tile([128, 1], f32, name="r", tag="r", bufs=4)
                                nc.vector.reciprocal(r[:], o_ps[ql][:, HD : HD + 1])
                                dst = ot[t][:, HD * h : HD * (h + 1)]
                                if last_hp and ql % 2 == 1:
                                    # the final normalize chain gates the
                                    # B->C PSUM pool swap; ACT is idle by
                                    # now so split it across two engines
                                    nc.scalar.activation(
                                        dst, o_ps[ql][:, 0:HD], AF.Copy,
                                        scale=r[:],
                                    )
                                else:
                                    nc.vector.tensor_scalar_mul(
                                        dst, o_ps[ql][:, 0:HD], r[:]
                                    )

                # ---------------- Phase C: o^T + final projection ----------
                oTa = [
                    pb.tile([128, T], bf16, name=f"oTa{j}", tag=f"oTa{j}")
                    for j in range(4)
                ]
                oTb = pb.tile([64, T], bf16, name="oTb", tag="oTb")
                wo_tiles = []
                for k in range(5):
                    rows = 128 if k < 4 else 64
                    wot_ = pb.tile([128, D], bf16, name=f"wo{k}", tag=f"wo{k}")
                    nc.sync.dma_start(
                        wot_[0:rows, :], woT[k * 128 : k * 128 + rows, :]
                    )
                    wo_tiles.append(wot_)
                with tc.tile_pool(name="pcps", bufs=1, space="PSUM") as pcps:

                    def o_transp(t):
                        for j in range(4):
                            tp = pcps.tile(
                                [128, 128], bf16, name="tpo", tag="otp", bufs=2
                            )
                            nc.tensor.transpose(
                                tp[:],
                                ot[t][:, 128 * j : 128 * (j + 1)],
                                ident_bf[:],
                            )
                            nc.vector.tensor_copy(
                                oTa[j][:, t * 128 : (t + 1) * 128], tp[:]
                            )
                        tpb = pcps.tile([64, 128], bf16, name="tpb", tag="otp", bufs=2)
                        nc.tensor.transpose(
                            tpb[:],
                            ot[t][:, 512:DV],
                            ident_bf[:],
                        )
                        nc.vector.tensor_copy(
                            oTb[:, t * 128 : (t + 1) * 128], tpb[:]
                        )

                    def final(t):
                        fps = [
                            pcps.tile(
                                [128, 384], f32, name=f"fps{j3}", tag=f"f{j3}", bufs=2
                            )
                            for j3 in range(3)
                        ]
                        # k-outer / j3-inner: the 3 matmuls of each k share
                        # the o^T stationary, hiding its LDWEIGHTS
                        for k in range(5):
                            rows = 128 if k < 4 else 64
                            lhsT = (
                                oTa[k][:, t * 128 : (t + 1) * 128]
                                if k < 4
                                else oTb[:, t * 128 : (t + 1) * 128]
                            )
                            for j3 in range(3):
                                nc.tensor.matmul(
                                    fps[j3][:],
                                    lhsT,
                                    wo_tiles[k][0:rows, 384 * j3 : 384 * (j3 + 1)],
                                    start=(k == 0),
                                    stop=(k == 4),
                                )
                        if t == NT - 1:
                            # last tile: one contiguous staging tile and a
                            # single store descriptor — three serial
                            # descriptors (~630 ns each on Sync) otherwise
                            # lengthen the end-of-kernel tail
                            fbig = pb.tile(
                                [128, D], f32, name="fbig", tag="fbig", bufs=1
                            )
                            for j3 in range(3):
                                cp = (
                                    nc.vector.tensor_copy
                                    if j3 % 2
                                    else nc.scalar.copy
                                )
                                cp(
                                    fbig[:, 384 * j3 : 384 * (j3 + 1)],
                                    fps[j3][:],
                                )
                            nc.sync.dma_start(
                                out[t * 128 : (t + 1) * 128, :], fbig[:]
                            )
                            return
                        for j3 in range(3):
                            fout = pb.tile(
                                [128, 384], f32, name="fout", tag="fout", bufs=4
                            )
                            if (t * 3 + j3) % 2 == 1:
                                nc.vector.tensor_copy(fout[:], fps[j3][:])
                            else:
                                nc.scalar.copy(fout[:], fps[j3][:])
                            nc.sync.dma_start(
                                out[
                                    t * 128 : (t + 1) * 128,
                                    384 * j3 : 384 * (j3 + 1),
                                ],
                                fout[:],
                            )

                    o_transp(0)
                    for t in range(NT):
                        if t + 1 < NT:
                            o_transp(t + 1)
                        final(t)

    nc.compile()
    return nc


def get_nc(debug=False):
    key = bool(debug)
    if key not in _NC_CACHE:
        _NC_CACHE[key] = _build(debug)
    return _NC_CACHE[key]


def make_in_maps(x, cos, sin, Wq, Wk, Wv, Wo):
    import ml_dtypes

    x = np.asarray(x, np.float32)
    cos = np.asarray(cos, np.float32)
    sin = np.asarray(sin, np.float32)
    Wq, Wk, Wv, Wo = (np.asarray(w, np.float32) for w in (Wq, Wk, Wv, Wo))
    cos_bf = cos.astype(ml_dtypes.bfloat16)
    # sign-folded, partner-permuted sin: snP[t,i] = sin[t,(i+72)%144] * s,
    # s = +1 for i<72, -1 for i>=72; makes every rope combine a plain add
    snp = sin[:, (np.arange(HD) + 72) % HD].copy()
    snp[:, 72:] *= -1.0
    snp_bf = snp.astype(ml_dtypes.bfloat16)

    in_maps = []
    for c in range(NCORES):
        b, hg = divmod(c, 2)
        heads = [HL * hg + i for i in range(HL)]

        def qk_merged(Wq_, Wk_):
            # rows: [q a-blocks 4x128 | k a-blocks 4x128 | q b 4x16 | k b 4x16]
            Wsel = np.zeros((WQK, D), np.float32)
            for i, g in enumerate(heads):
                Wsel[128 * i : 128 * i + 128] = Wq_[144 * g : 144 * g + 128]
                Wsel[512 + 128 * i : 512 + 128 * i + 128] = Wk_[
                    144 * g : 144 * g + 128
                ]
                Wsel[1024 + 16 * i : 1024 + 16 * i + 16] = Wq_[
                    144 * g + 128 : 144 * g + 144
                ]
                Wsel[1088 + 16 * i : 1088 + 16 * i + 16] = Wk_[
                    144 * g + 128 : 144 * g + 144
                ]
            return np.ascontiguousarray(Wsel.T)

        wv_sel = np.concatenate([Wv[144 * g : 144 * g + 144] for g in heads], 0)
        wo_sel = np.concatenate([Wo[:, 144 * g : 144 * g + 144] for g in heads], 1)
        in_maps.append(
            {
                "xT": np.ascontiguousarray(x[b].T).astype(ml_dtypes.bfloat16),
                "wqkT": qk_merged(Wq, Wk).astype(ml_dtypes.bfloat16),
                "wvT": np.ascontiguousarray(wv_sel.T).astype(ml_dtypes.bfloat16),
                "woT": np.ascontiguousarray(wo_sel.T).astype(ml_dtypes.bfloat16),
                "cosN": cos_bf,
                "snPN": snp_bf,
                "identB": np.eye(128, dtype=ml_dtypes.bfloat16),
            }
        )
    return in_maps


def kernel(x, cos, sin, Wq, Wk, Wv, Wo, _trace=False, _trace_kwargs=None):
    from concourse.bass_utils import run_bass_kernel_spmd

    nc = get_nc()
    in_maps = make_in_maps(x, cos, sin, Wq, Wk, Wv, Wo)
    res = run_bass_kernel_spmd(
        nc,
        in_maps,
        list(range(NCORES)),
        trace=_trace,
        **(_trace_kwargs or {}),
    )
    parts = [res.results[c]["out"] for c in range(NCORES)]
    outb = np.stack([parts[2 * b] + parts[2 * b + 1] for b in range(B)])
    if _trace:
        kernel.last_results = res
    return outb.astype(np.float32)


# revision 22
# speedup vs baseline: 1.1610x; 1.1610x over previous
"""Trainium2 Bass kernel for a fused multi-head attention block.

Reference computation (B=4, T=2048, D=1152, H=8, HD=144, full rotary):
    q,k,v = x@Wq.T, x@Wk.T, x@Wv.T   (per head)
    q,k   = rope(q, k, cos, sin)
    o     = softmax(q k^T / sqrt(HD)) v
    out   = o @ Wo.T

Sharding (8 cores): core c = (batch b = c//2, head-group hg = c%2).
Each core computes 4 heads of one batch and a partial output
out_part = o_local @ Wo[:, hg_cols].T ; host sums the two partials per batch.

Per-core structure (v2 — rebuilt from trace analysis of v1, 476us -> 391us):
  * Projections: q and k are computed in ONE merged pass (wqkT [D,1152] =
    [q a-blocks 512 | k a-blocks 512 | q b-dims 64 | k b-dims 64]) with 3
    matmuls per (n,k-chunk) sharing the x stationary (LDW amortized 1:3).
  * rope reads the projection PSUM directly (no copy): 4 DVE muls using a
    host-precomputed sign-folded/permuted sin table (snP) make every
    combine a plain DVE add; combines write bf16 q/k tiles which PE
    transposes to [e,t].  Transpose results copy out on the Scalar engine
    (idle during phase A).
  * Inputs arrive as ONE DMA descriptor per logical transfer (the Sync
    engine serializes descriptors at ~630 ns — v1's ~90 descriptors gated
    the v projection).  x is piece-major (all 9 D-chunks of each 256-col
    t-piece together) and the first wave is chunk-granular so matmul
    (n=0,k) unblocks as pair k lands.  Dummy warmup matmuls keep HAM at
    K=8/8 through the initial DMA window.  The score-tail b-block replicas
    are DMAed in two column halves so the first half issues mid-phase-A.
  * Attention processes TWO 512-wide q-blocks at once per (head, kt):
    scores S^T [keys, 1024q] into a 2-bank PSUM tile.  Per key-tile: two
    K=32 b-block tails FIRST (start=True, distinct tile_position
    row-groups -> concurrent in PE), then two K=128 mains sharing the kTa
    stationary (LDWEIGHTS hidden), ONE [128,1024] Exp (halves the ACT
    fixed overhead that paced v1), then 8 PV matmuls into 3 PSUM banks of
    packed 145-wide accumulators (softmax denominator via ones column of
    v).  The kt loop is software-pipelined depth 2 (scores run two
    key-tiles ahead of PV) so the PE never idles on scores->exp->PV
    latency; O1 is double-buffered so the next head's PV doesn't wait on
    the normalize.  PSUM budget: 2x2-bank sps + 2+1+1 accumulator banks
    = 8 (the binding constraint throughout).
  * Final projection in bf16: o normalized straight to bf16, PE transpose,
    then k-outer/j3-inner matmuls sharing the o^T stationary 1:3.
  * dtypes: all matmuls bf16 (f32 accum in PSUM); output f32.
"""

import numpy as np

B, T, D, H = 4, 2048, 1152, 8
HL = 4              # heads per core
HD = 144            # head dim
DV = HL * HD        # 576, v/o width
WQK = 1152          # merged q/k projection width: 512 + 512 + 64 + 64
NT = T // 128       # 16 t-tiles
KC = D // 128       # 9 contraction chunks
SCALE = float(HD) ** -0.5
NCORES = 8

_NC_CACHE = {}


def _build(debug=False):
    import concourse.bacc as bacc
    import concourse.mybir as mybir
    from concourse.tile import TileContext

    dt = mybir.dt
    f32, bf16 = dt.float32, dt.bfloat16
    AF = mybir.ActivationFunctionType

    nc = bacc.Bacc(
        "TRN2",
        target_bir_lowering=False,
        debug=debug,
        enable_asserts=False,
        num_devices=NCORES,
    )

    xT = nc.declare_dram_parameter("xT", [D, T], bf16, isOutput=False)
    wqkT = nc.declare_dram_parameter("wqkT", [D, WQK], bf16, isOutput=False)
    wvT = nc.declare_dram_parameter("wvT", [D, DV], bf16, isOutput=False)
    woT = nc.declare_dram_parameter("woT", [DV, D], bf16, isOutput=False)
    cosN = nc.declare_dram_parameter("cosN", [T, HD], bf16, isOutput=False)
    snPN = nc.declare_dram_parameter("snPN", [T, HD], bf16, isOutput=False)
    identB = nc.declare_dram_parameter("identB", [128, 128], bf16, isOutput=False)
    out = nc.declare_dram_parameter("out", [T, D], f32, isOutput=True)

    with TileContext(nc) as tc:
        with tc.tile_pool(name="persist", bufs=1) as P0:
            ident_bf = P0.tile([128, 128], bf16, name="ident_bf", tag="ident_bf")
            nc.sync.dma_start(ident_bf[:], identB[:])

            qTa = [
                P0.tile([128, T], bf16, name=f"qTa{h}", tag=f"qTa{h}")
                for h in range(HL)
            ]
            kTa = [
                P0.tile([128, T], bf16, name=f"kTa{h}", tag=f"kTa{h}")
                for h in range(HL)
            ]
            qTB = P0.tile([128, T], bf16, name="qTB", tag="qTB")
            kTB = P0.tile([128, T], bf16, name="kTB", tag="kTB")
            # per-head replicas of the b-block rows at all four 32-row groups,
            # so four score-tail K=32 matmuls can issue to distinct PE
            # row-groups and overlap in the array
            qTBr = [
                P0.tile([128, T], bf16, name=f"qTBr{h}", tag=f"qTBr{h}")
                for h in range(HL)
            ]
            kTBr = [
                P0.tile([128, T], bf16, name=f"kTBr{h}", tag=f"kTBr{h}")
                for h in range(HL)
            ]
            vt = [
                P0.tile([128, HL * (HD + 1)], bf16, name=f"v{t}", tag=f"v{t}")
                for t in range(NT)
            ]

            # ---------------- Phase A: projections + rope + transposes -----
            with tc.tile_pool(name="pa", bufs=1) as pa:

                # single SBUF tiles with one DMA descriptor per logical
                # transfer (3D access patterns) — v1/v2 used ~90 descriptors
                # which serialized on the Sync engine (~630 ns each) and
                # gated the first half of the v projection
                xbig = pa.tile([128, KC * T], bf16, name="xbig", tag="xbig")
                x3 = xbig.rearrange("p (c t) -> p c t", c=KC)
                xs = xT.rearrange("(c p) t -> p c t", p=128)
                xtiles = [x3[:, k] for k in range(KC)]
                cos_sb = pa.tile([128, NT * HD], bf16, name="cos_sb", tag="cos_sb")
                snp_sb = pa.tile([128, NT * HD], bf16, name="snp_sb", tag="snp_sb")
                wvbig = pa.tile([128, KC * DV], bf16, name="wvbig", tag="wvbig")
                wv3 = wvbig.rearrange("p (c e) -> p c e", c=KC)
                wv_tiles = [wv3[:, k] for k in range(KC)]
                # first wave at chunk granularity (wv_k + x[k, piece0]
                # interleaved) so matmul (n=0,k) unblocks as pair k lands
                # instead of after the whole wv + piece transfers; later x
                # pieces are single descriptors (Sync-engine issue is
                # ~630 ns per descriptor)
                NP = 8
                PW = T // NP
                wvs = wvT.rearrange("(c p) e -> p c e", p=128)
                for k in range(KC):
                    nc.sync.dma_start(wv3[:, k], wvs[:, k])
                    nc.sync.dma_start(
                        x3[:, k, 0:PW], xs[:, k, 0:PW]
                    )
                for p in range(1, NP):
                    nc.sync.dma_start(
                        x3[:, :, p * PW : (p + 1) * PW],
                        xs[:, :, p * PW : (p + 1) * PW],
                    )

                # ---- v projection ----
                with tc.tile_pool(name="pavps", bufs=1, space="PSUM") as pavps:
                    # dummy matmuls bridge the initial DMA window so HAM
                    # reaches K=8/8 before the first real matmul
                    warm = pavps.tile(
                        [128, 128], f32, name="warm", tag="warm", bufs=1
                    )
                    with tc.high_priority():
                        for _ in range(10):
                            nc.tensor.matmul(
                                warm[:], ident_bf[:], ident_bf[:],
                                start=True, stop=True,
                            )
                    for n in range(NT):
                        ps_v = pavps.tile(
                            [128, DV], f32, name="ps_v", tag="pv", bufs=2
                        )
                        for k in range(KC):
                            st, sp = k == 0, k == KC - 1
                            lhs = xtiles[k][:, n * 128 : (n + 1) * 128]
                            nc.tensor.matmul(
                                ps_v[:, 0:512], lhs, wv_tiles[k][:, 0:512],
                                start=st, stop=sp,
                            )
                            nc.tensor.matmul(
                                ps_v[:, 512:DV], lhs, wv_tiles[k][:, 512:DV],
                                start=st, stop=sp,
                            )
                        v3 = vt[n].rearrange("p (h e) -> p h e", h=HL)
                        nc.vector.tensor_copy(
                            v3[:, :, 0:HD],
                            ps_v.rearrange("p (h e) -> p h e", h=HL),
                        )
                        nc.gpsimd.memset(v3[:, :, HD : HD + 1], 1.0)

                # ---- merged q/k projection weights + trig tables ----
                wqkbig = pa.tile([128, KC * WQK], bf16, name="wqkbig", tag="wqkbig")
                wqk3 = wqkbig.rearrange("p (c e) -> p c e", c=KC)
                wqk_tiles = [wqk3[:, k] for k in range(KC)]
                nc.sync.dma_start(
                    wqk3, wqkT.rearrange("(c p) e -> p c e", p=128)
                )
                nc.sync.dma_start(
                    cos_sb.rearrange("p (n r) -> p n r", n=NT),
                    cosN.rearrange("(n p) r -> p n r", p=128),
                )
                nc.sync.dma_start(
                    snp_sb.rearrange("p (n r) -> p n r", n=NT),
                    snPN.rearrange("(n p) r -> p n r", p=128),
                )

                def trig3(sb, n):
                    # [128, 144] row block for t-tile n, broadcast over 4 heads
                    return (
                        sb[:, n * HD : (n + 1) * HD]
                        .rearrange("p (o r) -> p o r", o=1)
                        .to_broadcast([128, HL, HD])
                    )

                def rope_one(ps_a, ps_bq, qtl, cos3, snp3):
                    """ps_a [128,512] f32 (4 a-blocks), ps_bq [128,4,16] f32
                    view of the packed b dims -> qtl [128,640] bf16 with
                    rotary applied (layout: 4x128 a-blocks | 4x(16+16pad)).

                    m1[j] = q[j]*cos[j]; m2[j] = q[j]*snP[j] where
                    snP[i] = sin[(i+72)%144] * (+1 if i<72 else -1), so every
                    combine is a plain add: out[j] = m1[j] + m2[(j+72)%144].
                    """
                    pa3 = ps_a.rearrange("p (h e) -> p h e", h=HL)
                    m1 = pa.tile([128, 576], bf16, name="m1", tag="m1", bufs=2)
                    m2 = pa.tile([128, 576], bf16, name="m2", tag="m2", bufs=2)
                    m1a = m1[:, 0:512].rearrange("p (h e) -> p h e", h=HL)
                    m1b = m1[:, 512:576].rearrange("p (h e) -> p h e", h=HL)
                    m2a = m2[:, 0:512].rearrange("p (h e) -> p h e", h=HL)
                    m2b = m2[:, 512:576].rearrange("p (h e) -> p h e", h=HL)
                    v = nc.vector
                    v.tensor_mul(m1a[:], pa3[:], cos3[:, :, 0:128])
                    v.tensor_mul(m1b[:], ps_bq[:], cos3[:, :, 128:144])
                    v.tensor_mul(m2a[:], pa3[:], snp3[:, :, 0:128])
                    v.tensor_mul(m2b[:], ps_bq[:], snp3[:, :, 128:144])
                    oa = qtl[:, 0:512].rearrange("p (h e) -> p h e", h=HL)
                    ob = qtl[:, 512:640].rearrange("p (h e) -> p h e", h=HL)
                    # all-bf16 SBUF operands -> DVE 4x fast mode
                    v.tensor_add(oa[:, :, 0:56], m1a[:, :, 0:56], m2a[:, :, 72:128])
                    v.tensor_add(oa[:, :, 56:72], m1a[:, :, 56:72], m2b[:, :, 0:16])
                    v.tensor_add(oa[:, :, 72:128], m1a[:, :, 72:128], m2a[:, :, 0:56])
                    v.tensor_add(ob[:, :, 0:16], m1b[:], m2a[:, :, 56:72])
                    nc.gpsimd.memset(ob[:, :, 16:32], 0.0)

                with tc.tile_pool(name="paqps", bufs=1, space="PSUM") as paqps:

                    def transposes(n, qtl, ktl):
                        for src, dsts, dstb in ((qtl, qTa, qTB), (ktl, kTa, kTB)):
                            for j in range(5):
                                tp = paqps.tile(
                                    [128, 128], bf16, name="tp", tag="tp", bufs=3
                                )
                                nc.tensor.transpose(
                                    tp[:], src[:, 128 * j : 128 * (j + 1)], ident_bf[:]
                                )
                                dst = dsts[j] if j < 4 else dstb
                                nc.scalar.copy(
                                    dst[:, n * 128 : (n + 1) * 128], tp[:]
                                )

                    pend = None
                    for n in range(NT):
                        ps_q = paqps.tile(
                            [128, 512], f32, name="ps_q", tag="psq", bufs=2
                        )
                        ps_k = paqps.tile(
                            [128, 512], f32, name="ps_k", tag="psk", bufs=2
                        )
                        ps_b = paqps.tile(
                            [128, 128], f32, name="ps_b", tag="psb", bufs=1
                        )
                        for k in range(KC):
                            st, sp = k == 0, k == KC - 1
                            lhs = xtiles[k][:, n * 128 : (n + 1) * 128]
                            nc.tensor.matmul(
                                ps_q[:], lhs, wqk_tiles[k][:, 0:512],
                                start=st, stop=sp,
                            )
                            nc.tensor.matmul(
                                ps_k[:], lhs, wqk_tiles[k][:, 512:1024],
                                start=st, stop=sp,
                            )
                            nc.tensor.matmul(
                                ps_b[:], lhs, wqk_tiles[k][:, 1024:1152],
                                start=st, stop=sp,
                            )
                        if pend is not None:
                            transposes(*pend)
                        qtl = pa.tile([128, 640], bf16, name="qtl", tag="qtl", bufs=2)
                        ktl = pa.tile([128, 640], bf16, name="ktl", tag="ktl", bufs=2)
                        c3, s3 = trig3(cos_sb, n), trig3(snp_sb, n)
                        rope_one(
                            ps_q,
                            ps_b[:, 0:64].rearrange("p (h e) -> p h e", h=HL),
                            qtl, c3, s3,
                        )
                        rope_one(
                            ps_k,
                            ps_b[:, 64:128].rearrange("p (h e) -> p h e", h=HL),
                            ktl, c3, s3,
                        )
                        pend = (n, qtl, ktl)
                    transposes(*pend)

                # replicate the b-blocks in two column halves so the
                # first half's DMAs issue as soon as transposes n<=7 are
                # done (subtile deps) instead of after the whole phase;
                # h-major so head 0 lands first
                for half in range(2):
                    cl, cr = half * 1024, (half + 1) * 1024
                    for hh in range(HL):
                        for j in range(4):
                            nc.sync.dma_start(
                                qTBr[hh][32 * j : 32 * j + 32, cl:cr],
                                qTB[32 * hh : 32 * hh + 32, cl:cr],
                            )
                            nc.sync.dma_start(
                                kTBr[hh][32 * j : 32 * j + 32, cl:cr],
                                kTB[32 * hh : 32 * hh + 32, cl:cr],
                            )

            # ---------------- Phase B: attention --------------------------
            with tc.tile_pool(name="pb", bufs=1) as pb:
                ot = [
                    pb.tile([128, DV], bf16, name=f"o{t}", tag=f"o{t}")
                    for t in range(NT)
                ]
                with tc.tile_pool(name="pbps", bufs=1, space="PSUM") as pbps:
                    HD1 = HD + 1
                    for p in range(2):
                        qof = p * 1024
                        for h in range(HL):
                            # 8 PV accumulators packed into 3 PSUM banks;
                            # O1 double-buffered so the next (h,p)'s first PV
                            # matmuls don't wait on this one's normalize
                            O1 = pbps.tile(
                                [128, 3 * HD1], f32, name="O1", tag="O1", bufs=2
                            )
                            O2 = pbps.tile(
                                [128, 3 * HD1], f32, name="O2", tag="O2", bufs=1
                            )
                            O3 = pbps.tile(
                                [128, 2 * HD1], f32, name="O3", tag="O3", bufs=1
                            )
                            o_ps = (
                                [O1[:, i * HD1 : (i + 1) * HD1] for i in range(3)]
                                + [O2[:, i * HD1 : (i + 1) * HD1] for i in range(3)]
                                + [O3[:, i * HD1 : (i + 1) * HD1] for i in range(2)]
                            )

                            def scores_one(kt):
                                # one key-tile into one 2-bank PSUM tile:
                                # per 512-q half, a K=32 b-block tail first
                                # (start=True, two tails on distinct PE
                                # row-groups run concurrently) then the
                                # K=128 main carrying stop; the two mains
                                # share the kTa stationary so its
                                # LDWEIGHTS is hidden. ONE [128,1024] exp.
                                sps = pbps.tile(
                                    [128, 1024], f32, name="sps", tag="sc", bufs=2
                                )
                                for half in range(2):
                                    rg = 2 * (kt % 2) + half
                                    nc.tensor.matmul(
                                        sps[:, 512 * half : 512 * (half + 1)],
                                        kTBr[h][
                                            32 * rg : 32 * rg + 32,
                                            kt * 128 : (kt + 1) * 128,
                                        ],
                                        qTBr[h][
                                            32 * rg : 32 * rg + 32,
                                            qof + 512 * half : qof + 512 * (half + 1),
                                        ],
                                        start=True,
                                        stop=False,
                                        tile_position=(32 * rg, 0),
                                    )
                                for half in range(2):
                                    nc.tensor.matmul(
                                        sps[:, 512 * half : 512 * (half + 1)],
                                        kTa[h][:, kt * 128 : (kt + 1) * 128],
                                        qTa[h][
                                            :,
                                            qof + 512 * half : qof + 512 * (half + 1),
                                        ],
                                        start=False,
                                        stop=True,
                                    )
                                E = pb.tile(
                                    [128, 1024], bf16, name="E", tag="E", bufs=6
                                )
                                nc.scalar.activation(
                                    E[:], sps[:], AF.Exp, scale=SCALE
                                )
                                return E

                            def pv_one(kt, E):
                                for ql in range(8):
                                    # start/stop are carried by the first/
                                    # last matmul touching each bank
                                    st = kt == 0 and ql in (0, 3, 6)
                                    sp = kt == NT - 1 and ql in (2, 5, 7)
                                    nc.tensor.matmul(
                                        o_ps[ql][:],
                                        E[:, ql * 128 : (ql + 1) * 128],
                                        vt[kt][:, HD1 * h : HD1 * (h + 1)],
                                        start=st,
                                        stop=sp,
                                    )

                            # software pipeline depth 2: scores run two
                            # key-tiles ahead of PV, so when EXP(kt)
                            # completes the PE has both PV(kt) and
                            # scores(kt+2) ready and never idles on the
                            # scores->exp->PV latency chain
                            eq = [scores_one(0), scores_one(1)]
                            for kt in range(NT):
                                if kt + 2 < NT:
                                    eq.append(scores_one(kt + 2))
                                pv_one(kt, eq.pop(0))
                            last_hp = p == 1 and h == HL - 1
                            for ql in range(8):
                                t = 8 * p + ql
                                r = pb.tile([128, 1], f32, name="r", tag="r", bufs=4)
                                nc.vector.reciprocal(r[:], o_ps[ql][:, HD : HD + 1])
                                dst = ot[t][:, HD * h : HD * (h + 1)]
                                if last_hp and ql % 2 == 1:
                                    # the final normalize chain gates the
                                    # B->C PSUM pool swap; ACT is idle by
                                    # now so split it across two engines
                                    nc.scalar.activation(
                                        dst, o_ps[ql][:, 0:HD], AF.Copy,
                                        scale=r[:],
                                    )
                                else:
                                    nc.vector.tensor_scalar_mul(
                                        dst, o_ps[ql][:, 0:HD], r[:]
                                    )

                # ---------------- Phase C: o^T + final projection ----------
                oTa = [
                    pb.tile([128, T], bf16, name=f"oTa{j}", tag=f"oTa{j}")
                    for j in range(4)
                ]
                oTb = pb.tile([64, T], bf16, name="oTb", tag="oTb")
                wo_tiles = []
                for k in range(5):
                    rows = 128 if k < 4 else 64
                    wot_ = pb.tile([128, D], bf16, name=f"wo{k}", tag=f"wo{k}")
                    nc.sync.dma_start(
                        wot_[0:rows, :], woT[k * 128 : k * 128 + rows, :]
                    )
                    wo_tiles.append(wot_)
                with tc.tile_pool(name="pcps", bufs=1, space="PSUM") as pcps:

                    def o_transp(t):
                        for j in range(4):
                            tp = pcps.tile(
                                [128, 128], bf16, name="tpo", tag="otp", bufs=2
                            )
                            nc.tensor.transpose(
                                tp[:],
                                ot[t][:, 128 * j : 128 * (j + 1)],
                                ident_bf[:],
                            )
                            nc.vector.tensor_copy(
                                oTa[j][:, t * 128 : (t + 1) * 128], tp[:]
                            )
                        tpb = pcps.tile([64, 128], bf16, name="tpb", tag="otp", bufs=2)
                        nc.tensor.transpose(
                            tpb[:],
                            ot[t][:, 512:DV],
                            ident_bf[:],
                        )
                        nc.vector.tensor_copy(
                            oTb[:, t * 128 : (t + 1) * 128], tpb[:]
                        )

                    def final(t):
                        fps = [
                            pcps.tile(
                                [128, 384], f32, name=f"fps{j3}", tag=f"f{j3}", bufs=2
                            )
                            for j3 in range(3)
                        ]
                        # k-outer / j3-inner: the 3 matmuls of each k share
                        # the o^T stationary, hiding its LDWEIGHTS
                        for k in range(5):
                            rows = 128 if k < 4 else 64
                            lhsT = (
                                oTa[k][:, t * 128 : (t + 1) * 128]
                                if k < 4
                                else oTb[:, t * 128 : (t + 1) * 128]
                            )
                            for j3 in range(3):
                                nc.tensor.matmul(
                                    fps[j3][:],
                                    lhsT,
                                    wo_tiles[k][0:rows, 384 * j3 : 384 * (j3 + 1)],
                                    start=(k == 0),
                                    stop=(k == 4),
                                )
                        for j3 in range(3):
                            fout = pb.tile(
                                [128, 384], f32, name="fout", tag="fout", bufs=4
                            )
                            if (t * 3 + j3) % 2 == 1:
                                nc.vector.tensor_copy(fout[:], fps[j3][:])
                            else:
                                nc.scalar.copy(fout[:], fps[j3][:])
                            nc.sync.dma_start(
                                out[
                                    t * 128 : (t + 1) * 128,
                                    384 * j3 : 384 * (j3 + 1),
                                ],
                                fout[:],
                            )

                    o_transp(0)
                    for t in range(NT):
                        if t + 1 < NT:
                            o_transp(t + 1)
                        final(t)

    nc.compile()
    return nc


def get_nc(debug=False):
    key = bool(debug)
    if key not in _NC_CACHE:
        _NC_CACHE[key] = _build(debug)
    return _NC_CACHE[key]


def make_in_maps(x, cos, sin, Wq, Wk, Wv, Wo):
    import ml_dtypes

    x = np.asarray(x, np.float32)
    cos = np.asarray(cos, np.float32)
    sin = np.asarray(sin, np.float32)
    Wq, Wk, Wv, Wo = (np.asarray(w, np.float32) for w in (Wq, Wk, Wv, Wo))
    cos_bf = cos.astype(ml_dtypes.bfloat16)
    # sign-folded, partner-permuted sin: snP[t,i] = sin[t,(i+72)%144] * s,
    # s = +1 for i<72, -1 for i>=72; makes every rope combine a plain add
    snp = sin[:, (np.arange(HD) + 72) % HD].copy()
    snp[:, 72:] *= -1.0
    snp_bf = snp.astype(ml_dtypes.bfloat16)

    in_maps = []
    for c in range(NCORES):
        b, hg = divmod(c, 2)
        heads = [HL * hg + i for i in range(HL)]

        def qk_merged(Wq_, Wk_):
            # rows: [q a-blocks 4x128 | k a-blocks 4x128 | q b 4x16 | k b 4x16]
            Wsel = np.zeros((WQK, D), np.float32)
            for i, g in enumerate(heads):
                Wsel[128 * i : 128 * i + 128] = Wq_[144 * g : 144 * g + 128]
                Wsel[512 + 128 * i : 512 + 128 * i + 128] = Wk_[
                    144 * g : 144 * g + 128
                ]
                Wsel[1024 + 16 * i : 1024 + 16 * i + 16] = Wq_[
                    144 * g + 128 : 144 * g + 144
                ]
                Wsel[1088 + 16 * i : 1088 + 16 * i + 16] = Wk_[
                    144 * g + 128 : 144 * g + 144
                ]
            return np.ascontiguousarray(Wsel.T)

        wv_sel = np.concatenate([Wv[144 * g : 144 * g + 144] for g in heads], 0)
        wo_sel = np.concatenate([Wo[:, 144 * g : 144 * g + 144] for g in heads], 1)
        in_maps.append(
            {
                "xT": np.ascontiguousarray(x[b].T).astype(ml_dtypes.bfloat16),
                "wqkT": qk_merged(Wq, Wk).astype(ml_dtypes.bfloat16),
                "wvT": np.ascontiguousarray(wv_sel.T).astype(ml_dtypes.bfloat16),
                "woT": np.ascontiguousarray(wo_sel.T).astype(ml_dtypes.bfloat16),
                "cosN": cos_bf,
                "snPN": snp_bf,
                "identB": np.eye(128, dtype=ml_dtypes.bfloat16),
            }
        )
    return in_maps


def kernel(x, cos, sin, Wq, Wk, Wv, Wo, _trace=False, _trace_kwargs=None):
    from concourse.bass_utils import run_bass_kernel_spmd

    nc = get_nc()
    in_maps = make_in_maps(x, cos, sin, Wq, Wk, Wv, Wo)
    res = run_bass_kernel_spmd(
        nc,
        in_maps,
        list(range(NCORES)),
        trace=_trace,
        **(_trace_kwargs or {}),
    )
    parts = [res.results[c]["out"] for c in range(NCORES)]
    outb = np.stack([parts[2 * b] + parts[2 * b + 1] for b in range(B)])
    if _trace:
        kernel.last_results = res
    return outb.astype(np.float32)


# revision 23
# speedup vs baseline: 1.1852x; 1.0209x over previous
"""Trainium2 Bass kernel for a fused multi-head attention block.

Reference computation (B=4, T=2048, D=1152, H=8, HD=144, full rotary):
    q,k,v = x@Wq.T, x@Wk.T, x@Wv.T   (per head)
    q,k   = rope(q, k, cos, sin)
    o     = softmax(q k^T / sqrt(HD)) v
    out   = o @ Wo.T

Sharding (8 cores): core c = (batch b = c//2, head-group hg = c%2).
Each core computes 4 heads of one batch and a partial output
out_part = o_local @ Wo[:, hg_cols].T ; host sums the two partials per batch.

Per-core structure (v2 — rebuilt from trace analysis of v1, 476us -> 391us):
  * Projections: q and k are computed in ONE merged pass (wqkT [D,1152] =
    [q a-blocks 512 | k a-blocks 512 | q b-dims 64 | k b-dims 64]) with 3
    matmuls per (n,k-chunk) sharing the x stationary (LDW amortized 1:3).
  * rope reads the projection PSUM directly (no copy): 4 DVE muls using a
    host-precomputed sign-folded/permuted sin table (snP) make every
    combine a plain DVE add; combines write bf16 q/k tiles which PE
    transposes to [e,t].  Transpose results copy out on the Scalar engine
    (idle during phase A).
  * Inputs arrive as ONE DMA descriptor per logical transfer (the Sync
    engine serializes descriptors at ~630 ns — v1's ~90 descriptors gated
    the v projection).  x is piece-major (all 9 D-chunks of each 256-col
    t-piece together) and the first wave is chunk-granular so matmul
    (n=0,k) unblocks as pair k lands.  Dummy warmup matmuls keep HAM at
    K=8/8 through the initial DMA window.  The score-tail b-block replicas
    are DMAed in two column halves so the first half issues mid-phase-A.
  * Attention processes TWO 512-wide q-blocks at once per (head, kt):
    scores S^T [keys, 1024q] into a 2-bank PSUM tile.  Per key-tile: two
    K=32 b-block tails FIRST (start=True, distinct tile_position
    row-groups -> concurrent in PE), then two K=128 mains sharing the kTa
    stationary (LDWEIGHTS hidden), ONE [128,1024] Exp (halves the ACT
    fixed overhead that paced v1), then 8 PV matmuls into 3 PSUM banks of
    packed 145-wide accumulators (softmax denominator via ones column of
    v).  The kt loop is software-pipelined depth 2 (scores run two
    key-tiles ahead of PV) so the PE never idles on scores->exp->PV
    latency; O1 is double-buffered so the next head's PV doesn't wait on
    the normalize.  PSUM budget: 2x2-bank sps + 2+1+1 accumulator banks
    = 8 (the binding constraint throughout).
  * Final projection in bf16: o normalized straight to bf16, PE transpose,
    then k-outer/j3-inner matmuls sharing the o^T stationary 1:3.
  * dtypes: all matmuls bf16 (f32 accum in PSUM); output f32.
"""

import numpy as np

B, T, D, H = 4, 2048, 1152, 8
HL = 4              # heads per core
HD = 144            # head dim
DV = HL * HD        # 576, v/o width
WQK = 1152          # merged q/k projection width: 512 + 512 + 64 + 64
NT = T // 128       # 16 t-tiles
KC = D // 128       # 9 contraction chunks
SCALE = float(HD) ** -0.5
NCORES = 8

_NC_CACHE = {}


def _build(debug=False):
    import concourse.bacc as bacc
    import concourse.mybir as mybir
    from concourse.tile import TileContext

    dt = mybir.dt
    f32, bf16 = dt.float32, dt.bfloat16
    AF = mybir.ActivationFunctionType

    nc = bacc.Bacc(
        "TRN2",
        target_bir_lowering=False,
        debug=debug,
        enable_asserts=False,
        num_devices=NCORES,
    )

    xT = nc.declare_dram_parameter("xT", [D, T], bf16, isOutput=False)
    wqkT = nc.declare_dram_parameter("wqkT", [D, WQK], bf16, isOutput=False)
    wvT = nc.declare_dram_parameter("wvT", [D, DV], bf16, isOutput=False)
    woT = nc.declare_dram_parameter("woT", [DV, D], bf16, isOutput=False)
    cosN = nc.declare_dram_parameter("cosN", [T, HD], bf16, isOutput=False)
    snPN = nc.declare_dram_parameter("snPN", [T, HD], bf16, isOutput=False)
    identB = nc.declare_dram_parameter("identB", [128, 128], bf16, isOutput=False)
    out = nc.declare_dram_parameter("out", [T, D], f32, isOutput=True)

    with TileContext(nc) as tc:
        with tc.tile_pool(name="persist", bufs=1) as P0:
            ident_bf = P0.tile([128, 128], bf16, name="ident_bf", tag="ident_bf")
            nc.sync.dma_start(ident_bf[:], identB[:])

            qTa = [
                P0.tile([128, T], bf16, name=f"qTa{h}", tag=f"qTa{h}")
                for h in range(HL)
            ]
            kTa = [
                P0.tile([128, T], bf16, name=f"kTa{h}", tag=f"kTa{h}")
                for h in range(HL)
            ]
            qTB = P0.tile([128, T], bf16, name="qTB", tag="qTB")
            kTB = P0.tile([128, T], bf16, name="kTB", tag="kTB")
            # per-head replicas of the b-block rows at all four 32-row groups,
            # so four score-tail K=32 matmuls can issue to distinct PE
            # row-groups and overlap in the array
            qTBr = [
                P0.tile([128, T], bf16, name=f"qTBr{h}", tag=f"qTBr{h}")
                for h in range(HL)
            ]
            kTBr = [
                P0.tile([128, T], bf16, name=f"kTBr{h}", tag=f"kTBr{h}")
                for h in range(HL)
            ]
            vt = [
                P0.tile([128, HL * (HD + 1)], bf16, name=f"v{t}", tag=f"v{t}")
                for t in range(NT)
            ]

            # ---------------- Phase A: projections + rope + transposes -----
            with tc.tile_pool(name="pa", bufs=1) as pa:

                # single SBUF tiles with one DMA descriptor per logical
                # transfer (3D access patterns) — v1/v2 used ~90 descriptors
                # which serialized on the Sync engine (~630 ns each) and
                # gated the first half of the v projection
                xbig = pa.tile([128, KC * T], bf16, name="xbig", tag="xbig")
                x3 = xbig.rearrange("p (c t) -> p c t", c=KC)
                xs = xT.rearrange("(c p) t -> p c t", p=128)
                xtiles = [x3[:, k] for k in range(KC)]
                cos_sb = pa.tile([128, NT * HD], bf16, name="cos_sb", tag="cos_sb")
                snp_sb = pa.tile([128, NT * HD], bf16, name="snp_sb", tag="snp_sb")
                wvbig = pa.tile([128, KC * DV], bf16, name="wvbig", tag="wvbig")
                wv3 = wvbig.rearrange("p (c e) -> p c e", c=KC)
                wv_tiles = [wv3[:, k] for k in range(KC)]
                # first wave at chunk granularity (wv_k + x[k, piece0]
                # interleaved) so matmul (n=0,k) unblocks as pair k lands
                # instead of after the whole wv + piece transfers; later x
                # pieces are single descriptors (Sync-engine issue is
                # ~630 ns per descriptor)
                NP = 8
                PW = T // NP
                wvs = wvT.rearrange("(c p) e -> p c e", p=128)
                for k in range(KC):
                    nc.sync.dma_start(wv3[:, k], wvs[:, k])
                    nc.sync.dma_start(
                        x3[:, k, 0:PW], xs[:, k, 0:PW]
                    )
                for p in range(1, NP):
                    nc.sync.dma_start(
                        x3[:, :, p * PW : (p + 1) * PW],
                        xs[:, :, p * PW : (p + 1) * PW],
                    )

                # ---- v projection ----
                with tc.tile_pool(name="pavps", bufs=1, space="PSUM") as pavps:
                    # dummy matmuls bridge the initial DMA window so HAM
                    # reaches K=8/8 before the first real matmul
                    warm = pavps.tile(
                        [128, 128], f32, name="warm", tag="warm", bufs=1
                    )
                    with tc.high_priority():
                        for _ in range(10):
                            nc.tensor.matmul(
                                warm[:], ident_bf[:], ident_bf[:],
                                start=True, stop=True,
                            )
                    for n in range(NT):
                        ps_v = pavps.tile(
                            [128, DV], f32, name="ps_v", tag="pv", bufs=2
                        )
                        for k in range(KC):
                            st, sp = k == 0, k == KC - 1
                            lhs = xtiles[k][:, n * 128 : (n + 1) * 128]
                            nc.tensor.matmul(
                                ps_v[:, 0:512], lhs, wv_tiles[k][:, 0:512],
                                start=st, stop=sp,
                            )
                            nc.tensor.matmul(
                                ps_v[:, 512:DV], lhs, wv_tiles[k][:, 512:DV],
                                start=st, stop=sp,
                            )
                        v3 = vt[n].rearrange("p (h e) -> p h e", h=HL)
                        nc.vector.tensor_copy(
                            v3[:, :, 0:HD],
                            ps_v.rearrange("p (h e) -> p h e", h=HL),
                        )
                        nc.gpsimd.memset(v3[:, :, HD : HD + 1], 1.0)

                # ---- merged q/k projection weights + trig tables ----
                wqkbig = pa.tile([128, KC * WQK], bf16, name="wqkbig", tag="wqkbig")
                wqk3 = wqkbig.rearrange("p (c e) -> p c e", c=KC)
                wqk_tiles = [wqk3[:, k] for k in range(KC)]
                nc.sync.dma_start(
                    wqk3, wqkT.rearrange("(c p) e -> p c e", p=128)
                )
                nc.sync.dma_start(
                    cos_sb.rearrange("p (n r) -> p n r", n=NT),
                    cosN.rearrange("(n p) r -> p n r", p=128),
                )
                nc.sync.dma_start(
                    snp_sb.rearrange("p (n r) -> p n r", n=NT),
                    snPN.rearrange("(n p) r -> p n r", p=128),
                )

                def trig3(sb, n):
                    # [128, 144] row block for t-tile n, broadcast over 4 heads
                    return (
                        sb[:, n * HD : (n + 1) * HD]
                        .rearrange("p (o r) -> p o r", o=1)
                        .to_broadcast([128, HL, HD])
                    )

                def rope_one(ps_a, ps_bq, qtl, cos3, snp3):
                    """ps_a [128,512] f32 (4 a-blocks), ps_bq [128,4,16] f32
                    view of the packed b dims -> qtl [128,640] bf16 with
                    rotary applied (layout: 4x128 a-blocks | 4x(16+16pad)).

                    m1[j] = q[j]*cos[j]; m2[j] = q[j]*snP[j] where
                    snP[i] = sin[(i+72)%144] * (+1 if i<72 else -1), so every
                    combine is a plain add: out[j] = m1[j] + m2[(j+72)%144].
                    """
                    pa3 = ps_a.rearrange("p (h e) -> p h e", h=HL)
                    m1 = pa.tile([128, 576], bf16, name="m1", tag="m1", bufs=2)
                    m2 = pa.tile([128, 576], bf16, name="m2", tag="m2", bufs=2)
                    m1a = m1[:, 0:512].rearrange("p (h e) -> p h e", h=HL)
                    m1b = m1[:, 512:576].rearrange("p (h e) -> p h e", h=HL)
                    m2a = m2[:, 0:512].rearrange("p (h e) -> p h e", h=HL)
                    m2b = m2[:, 512:576].rearrange("p (h e) -> p h e", h=HL)
                    v = nc.vector
                    v.tensor_mul(m1a[:], pa3[:], cos3[:, :, 0:128])
                    v.tensor_mul(m1b[:], ps_bq[:], cos3[:, :, 128:144])
                    v.tensor_mul(m2a[:], pa3[:], snp3[:, :, 0:128])
                    v.tensor_mul(m2b[:], ps_bq[:], snp3[:, :, 128:144])
                    oa = qtl[:, 0:512].rearrange("p (h e) -> p h e", h=HL)
                    ob = qtl[:, 512:640].rearrange("p (h e) -> p h e", h=HL)
                    # all-bf16 SBUF operands -> DVE 4x fast mode
                    v.tensor_add(oa[:, :, 0:56], m1a[:, :, 0:56], m2a[:, :, 72:128])
                    v.tensor_add(oa[:, :, 56:72], m1a[:, :, 56:72], m2b[:, :, 0:16])
                    v.tensor_add(oa[:, :, 72:128], m1a[:, :, 72:128], m2a[:, :, 0:56])
                    v.tensor_add(ob[:, :, 0:16], m1b[:], m2a[:, :, 56:72])
                    nc.gpsimd.memset(ob[:, :, 16:32], 0.0)

                with tc.tile_pool(name="paqps", bufs=1, space="PSUM") as paqps:

                    def transposes(n, qtl, ktl):
                        for src, dsts, dstb in ((qtl, qTa, qTB), (ktl, kTa, kTB)):
                            for j in range(5):
                                tp = paqps.tile(
                                    [128, 128], bf16, name="tp", tag="tp", bufs=3
                                )
                                nc.tensor.transpose(
                                    tp[:], src[:, 128 * j : 128 * (j + 1)], ident_bf[:]
                                )
                                dst = dsts[j] if j < 4 else dstb
                                nc.scalar.copy(
                                    dst[:, n * 128 : (n + 1) * 128], tp[:]
                                )

                    pend = None
                    for n in range(NT):
                        ps_q = paqps.tile(
                            [128, 512], f32, name="ps_q", tag="psq", bufs=2
                        )
                        ps_k = paqps.tile(
                            [128, 512], f32, name="ps_k", tag="psk", bufs=2
                        )
                        ps_b = paqps.tile(
                            [128, 128], f32, name="ps_b", tag="psb", bufs=1
                        )
                        for k in range(KC):
                            st, sp = k == 0, k == KC - 1
                            lhs = xtiles[k][:, n * 128 : (n + 1) * 128]
                            nc.tensor.matmul(
                                ps_q[:], lhs, wqk_tiles[k][:, 0:512],
                                start=st, stop=sp,
                            )
                            nc.tensor.matmul(
                                ps_k[:], lhs, wqk_tiles[k][:, 512:1024],
                                start=st, stop=sp,
                            )
                            nc.tensor.matmul(
                                ps_b[:], lhs, wqk_tiles[k][:, 1024:1152],
                                start=st, stop=sp,
                            )
                        if pend is not None:
                            transposes(*pend)
                        qtl = pa.tile([128, 640], bf16, name="qtl", tag="qtl", bufs=2)
                        ktl = pa.tile([128, 640], bf16, name="ktl", tag="ktl", bufs=2)
                        c3, s3 = trig3(cos_sb, n), trig3(snp_sb, n)
                        rope_one(
                            ps_q,
                            ps_b[:, 0:64].rearrange("p (h e) -> p h e", h=HL),
                            qtl, c3, s3,
                        )
                        rope_one(
                            ps_k,
                            ps_b[:, 64:128].rearrange("p (h e) -> p h e", h=HL),
                            ktl, c3, s3,
                        )
                        pend = (n, qtl, ktl)
                    transposes(*pend)

                # replicate the b-blocks in two column halves so the
                # first half's DMAs issue as soon as transposes n<=7 are
                # done (subtile deps) instead of after the whole phase;
                # h-major so head 0 lands first
                for half in range(2):
                    cl, cr = half * 1024, (half + 1) * 1024
                    for hh in range(HL):
                        for j in range(4):
                            nc.sync.dma_start(
                                qTBr[hh][32 * j : 32 * j + 32, cl:cr],
                                qTB[32 * hh : 32 * hh + 32, cl:cr],
                            )
                            nc.sync.dma_start(
                                kTBr[hh][32 * j : 32 * j + 32, cl:cr],
                                kTB[32 * hh : 32 * hh + 32, cl:cr],
                            )

            # ---------------- Phase B: attention --------------------------
            with tc.tile_pool(name="pb", bufs=1) as pb:
                ot = [
                    pb.tile([128, DV], bf16, name=f"o{t}", tag=f"o{t}")
                    for t in range(NT)
                ]
                with tc.tile_pool(name="pbps", bufs=1, space="PSUM") as pbps:
                    HD1 = HD + 1
                    NITEM = 2 * HL * NT  # flat (q-pair, head, key-tile) items

                    def decode(idx):
                        p, rem = divmod(idx, HL * NT)
                        h, kt = divmod(rem, NT)
                        return p, h, kt

                    def scores_for(idx):
                        # one key-tile into one 2-bank PSUM tile: per 512-q
                        # half, a K=32 b-block tail first (start=True, two
                        # tails on distinct PE row-groups run concurrently)
                        # then the K=128 main carrying stop; the two mains
                        # share the kTa stationary so its LDWEIGHTS is
                        # hidden. ONE [128,1024] exp.
                        p, h, kt = decode(idx)
                        qof = p * 1024
                        sps = pbps.tile(
                            [128, 1024], f32, name="sps", tag="sc", bufs=2
                        )
                        for half in range(2):
                            rg = 2 * (kt % 2) + half
                            nc.tensor.matmul(
                                sps[:, 512 * half : 512 * (half + 1)],
                                kTBr[h][
                                    32 * rg : 32 * rg + 32,
                                    kt * 128 : (kt + 1) * 128,
                                ],
                                qTBr[h][
                                    32 * rg : 32 * rg + 32,
                                    qof + 512 * half : qof + 512 * (half + 1),
                                ],
                                start=True,
                                stop=False,
                                tile_position=(32 * rg, 0),
                            )
                        for half in range(2):
                            nc.tensor.matmul(
                                sps[:, 512 * half : 512 * (half + 1)],
                                kTa[h][:, kt * 128 : (kt + 1) * 128],
                                qTa[h][
                                    :,
                                    qof + 512 * half : qof + 512 * (half + 1),
                                ],
                                start=False,
                                stop=True,
                            )
                        E = pb.tile(
                            [128, 1024], bf16, name="E", tag="E", bufs=6
                        )
                        nc.scalar.activation(E[:], sps[:], AF.Exp, scale=SCALE)
                        return E

                    # ONE software pipeline (depth 2) across ALL (p,h,kt)
                    # items: scores prefetch crosses (head, q-pair)
                    # boundaries, so the scores->exp->PV chain never
                    # drains/refills between heads (the per-head pipeline
                    # paid an ~1us exp-latency bubble at each of the 8
                    # boundaries)
                    eq = [scores_for(0), scores_for(1)]
                    o_ps = None
                    for idx in range(NITEM):
                        p, h, kt = decode(idx)
                        if kt == 0:
                            # 8 PV accumulators packed into 3 PSUM banks;
                            # O1 double-buffered so this head's first PV
                            # matmuls don't wait on the previous normalize
                            O1 = pbps.tile(
                                [128, 3 * HD1], f32, name="O1", tag="O1", bufs=2
                            )
                            O2 = pbps.tile(
                                [128, 3 * HD1], f32, name="O2", tag="O2", bufs=1
                            )
                            O3 = pbps.tile(
                                [128, 2 * HD1], f32, name="O3", tag="O3", bufs=1
                            )
                            o_ps = (
                                [O1[:, i * HD1 : (i + 1) * HD1] for i in range(3)]
                                + [O2[:, i * HD1 : (i + 1) * HD1] for i in range(3)]
                                + [O3[:, i * HD1 : (i + 1) * HD1] for i in range(2)]
                            )
                        if idx + 2 < NITEM:
                            eq.append(scores_for(idx + 2))
                        E = eq.pop(0)
                        for ql in range(8):
                            # start/stop are carried by the first/last
                            # matmul touching each bank
                            st = kt == 0 and ql in (0, 3, 6)
                            sp = kt == NT - 1 and ql in (2, 5, 7)
                            nc.tensor.matmul(
                                o_ps[ql][:],
                                E[:, ql * 128 : (ql + 1) * 128],
                                vt[kt][:, HD1 * h : HD1 * (h + 1)],
                                start=st,
                                stop=sp,
                            )
                        if kt == NT - 1:
                            last_hp = idx == NITEM - 1
                            for ql in range(8):
                                t = 8 * p + ql
                                r = pb.tile(
                                    [128, 1], f32, name="r", tag="r", bufs=4
                                )
                                nc.vector.reciprocal(
                                    r[:], o_ps[ql][:, HD : HD + 1]
                                )
                                dst = ot[t][:, HD * h : HD * (h + 1)]
                                if last_hp and ql % 2 == 1:
                                    # the final normalize chain gates the
                                    # B->C PSUM pool swap; ACT is idle by
                                    # now so split it across two engines
                                    nc.scalar.activation(
                                        dst, o_ps[ql][:, 0:HD], AF.Copy,
                                        scale=r[:],
                                    )
                                else:
                                    nc.vector.tensor_scalar_mul(
                                        dst, o_ps[ql][:, 0:HD], r[:]
                                    )

                # ---------------- Phase C: o^T + final projection ----------
                oTa = [
                    pb.tile([128, T], bf16, name=f"oTa{j}", tag=f"oTa{j}")
                    for j in range(4)
                ]
                oTb = pb.tile([64, T], bf16, name="oTb", tag="oTb")
                wo_tiles = []
                for k in range(5):
                    rows = 128 if k < 4 else 64
                    wot_ = pb.tile([128, D], bf16, name=f"wo{k}", tag=f"wo{k}")
                    nc.sync.dma_start(
                        wot_[0:rows, :], woT[k * 128 : k * 128 + rows, :]
                    )
                    wo_tiles.append(wot_)
                with tc.tile_pool(name="pcps", bufs=1, space="PSUM") as pcps:

                    def o_transp(t):
                        for j in range(4):
                            tp = pcps.tile(
                                [128, 128], bf16, name="tpo", tag="otp", bufs=2
                            )
                            nc.tensor.transpose(
                                tp[:],
                                ot[t][:, 128 * j : 128 * (j + 1)],
                                ident_bf[:],
                            )
                            nc.vector.tensor_copy(
                                oTa[j][:, t * 128 : (t + 1) * 128], tp[:]
                            )
                        tpb = pcps.tile([64, 128], bf16, name="tpb", tag="otp", bufs=2)
                        nc.tensor.transpose(
                            tpb[:],
                            ot[t][:, 512:DV],
                            ident_bf[:],
                        )
                        nc.vector.tensor_copy(
                            oTb[:, t * 128 : (t + 1) * 128], tpb[:]
                        )

                    def final(t):
                        fps = [
                            pcps.tile(
                                [128, 384], f32, name=f"fps{j3}", tag=f"f{j3}", bufs=2
                            )
                            for j3 in range(3)
                        ]
                        # k-outer / j3-inner: the 3 matmuls of each k share
                        # the o^T stationary, hiding its LDWEIGHTS
                        for k in range(5):
                            rows = 128 if k < 4 else 64
                            lhsT = (
                                oTa[k][:, t * 128 : (t + 1) * 128]
                                if k < 4
                                else oTb[:, t * 128 : (t + 1) * 128]
                            )
                            for j3 in range(3):
                                nc.tensor.matmul(
                                    fps[j3][:],
                                    lhsT,
                                    wo_tiles[k][0:rows, 384 * j3 : 384 * (j3 + 1)],
                                    start=(k == 0),
                                    stop=(k == 4),
                                )
                        for j3 in range(3):
                            fout = pb.tile(
                                [128, 384], f32, name="fout", tag="fout", bufs=4
                            )
                            if (t * 3 + j3) % 2 == 1:
                                nc.vector.tensor_copy(fout[:], fps[j3][:])
                            else:
                                nc.scalar.copy(fout[:], fps[j3][:])
                            nc.sync.dma_start(
                                out[
                                    t * 128 : (t + 1) * 128,
                                    384 * j3 : 384 * (j3 + 1),
                                ],
                                fout[:],
                            )

                    o_transp(0)
                    for t in range(NT):
                        if t + 1 < NT:
                            o_transp(t + 1)
                        final(t)

    nc.compile()
    return nc


def get_nc(debug=False):
    key = bool(debug)
    if key not in _NC_CACHE:
        _NC_CACHE[key] = _build(debug)
    return _NC_CACHE[key]


def make_in_maps(x, cos, sin, Wq, Wk, Wv, Wo):
    import ml_dtypes

    x = np.asarray(x, np.float32)
    cos = np.asarray(cos, np.float32)
    sin = np.asarray(sin, np.float32)
    Wq, Wk, Wv, Wo = (np.asarray(w, np.float32) for w in (Wq, Wk, Wv, Wo))
    cos_bf = cos.astype(ml_dtypes.bfloat16)
    # sign-folded, partner-permuted sin: snP[t,i] = sin[t,(i+72)%144] * s,
    # s = +1 for i<72, -1 for i>=72; makes every rope combine a plain add
    snp = sin[:, (np.arange(HD) + 72) % HD].copy()
    snp[:, 72:] *= -1.0
    snp_bf = snp.astype(ml_dtypes.bfloat16)

    in_maps = []
    for c in range(NCORES):
        b, hg = divmod(c, 2)
        heads = [HL * hg + i for i in range(HL)]

        def qk_merged(Wq_, Wk_):
            # rows: [q a-blocks 4x128 | k a-blocks 4x128 | q b 4x16 | k b 4x16]
            Wsel = np.zeros((WQK, D), np.float32)
            for i, g in enumerate(heads):
                Wsel[128 * i : 128 * i + 128] = Wq_[144 * g : 144 * g + 128]
                Wsel[512 + 128 * i : 512 + 128 * i + 128] = Wk_[
                    144 * g : 144 * g + 128
                ]
                Wsel[1024 + 16 * i : 1024 + 16 * i + 16] = Wq_[
                    144 * g + 128 : 144 * g + 144
                ]
                Wsel[1088 + 16 * i : 1088 + 16 * i + 16] = Wk_[
                    144 * g + 128 : 144 * g + 144
                ]
            return np.ascontiguousarray(Wsel.T)

        wv_sel = np.concatenate([Wv[144 * g : 144 * g + 144] for g in heads], 0)
        wo_sel = np.concatenate([Wo[:, 144 * g : 144 * g + 144] for g in heads], 1)
        in_maps.append(
            {
                "xT": np.ascontiguousarray(x[b].T).astype(ml_dtypes.bfloat16),
                "wqkT": qk_merged(Wq, Wk).astype(ml_dtypes.bfloat16),
                "wvT": np.ascontiguousarray(wv_sel.T).astype(ml_dtypes.bfloat16),
                "woT": np.ascontiguousarray(wo_sel.T).astype(ml_dtypes.bfloat16),
                "cosN": cos_bf,
                "snPN": snp_bf,
                "identB": np.eye(128, dtype=ml_dtypes.bfloat16),
            }
        )
    return in_maps


def kernel(x, cos, sin, Wq, Wk, Wv, Wo, _trace=False, _trace_kwargs=None):
    from concourse.bass_utils import run_bass_kernel_spmd

    nc = get_nc()
    in_maps = make_in_maps(x, cos, sin, Wq, Wk, Wv, Wo)
    res = run_bass_kernel_spmd(
        nc,
        in_maps,
        list(range(NCORES)),
        trace=_trace,
        **(_trace_kwargs or {}),
    )
    parts = [res.results[c]["out"] for c in range(NCORES)]
    outb = np.stack([parts[2 * b] + parts[2 * b + 1] for b in range(B)])
    if _trace:
        kernel.last_results = res
    return outb.astype(np.float32)


# revision 24
# speedup vs baseline: 1.1865x; 1.0011x over previous
"""Trainium2 Bass kernel for a fused multi-head attention block.

Reference computation (B=4, T=2048, D=1152, H=8, HD=144, full rotary):
    q,k,v = x@Wq.T, x@Wk.T, x@Wv.T   (per head)
    q,k   = rope(q, k, cos, sin)
    o     = softmax(q k^T / sqrt(HD)) v
    out   = o @ Wo.T

Sharding (8 cores): core c = (batch b = c//2, head-group hg = c%2).
Each core computes 4 heads of one batch and a partial output
out_part = o_local @ Wo[:, hg_cols].T ; host sums the two partials per batch.

Per-core structure (v2 — rebuilt from trace analysis of v1, 476us -> 391us):
  * Projections: q and k are computed in ONE merged pass (wqkT [D,1152] =
    [q a-blocks 512 | k a-blocks 512 | q b-dims 64 | k b-dims 64]) with 3
    matmuls per (n,k-chunk) sharing the x stationary (LDW amortized 1:3).
  * rope reads the projection PSUM directly (no copy): 4 DVE muls using a
    host-precomputed sign-folded/permuted sin table (snP) make every
    combine a plain DVE add; combines write bf16 q/k tiles which PE
    transposes to [e,t].  Transpose results copy out on the Scalar engine
    (idle during phase A).
  * Inputs arrive as ONE DMA descriptor per logical transfer (the Sync
    engine serializes descriptors at ~630 ns — v1's ~90 descriptors gated
    the v projection).  x is piece-major (all 9 D-chunks of each 256-col
    t-piece together) and the first wave is chunk-granular so matmul
    (n=0,k) unblocks as pair k lands.  Dummy warmup matmuls keep HAM at
    K=8/8 through the initial DMA window.  The score-tail b-block replicas
    are DMAed in two column halves so the first half issues mid-phase-A.
  * Attention processes TWO 512-wide q-blocks at once per (head, kt):
    scores S^T [keys, 1024q] into a 2-bank PSUM tile.  Per key-tile: two
    K=32 b-block tails FIRST (start=True, distinct tile_position
    row-groups -> concurrent in PE), then two K=128 mains sharing the kTa
    stationary (LDWEIGHTS hidden), ONE [128,1024] Exp (halves the ACT
    fixed overhead that paced v1), then 8 PV matmuls into 3 PSUM banks of
    packed 145-wide accumulators (softmax denominator via ones column of
    v).  The kt loop is software-pipelined depth 2 (scores run two
    key-tiles ahead of PV) so the PE never idles on scores->exp->PV
    latency; O1 is double-buffered so the next head's PV doesn't wait on
    the normalize.  PSUM budget: 2x2-bank sps + 2+1+1 accumulator banks
    = 8 (the binding constraint throughout).
  * Final projection in bf16: o normalized straight to bf16, PE transpose,
    then k-outer/j3-inner matmuls sharing the o^T stationary 1:3.
  * dtypes: all matmuls bf16 (f32 accum in PSUM); output f32.
"""

import numpy as np

B, T, D, H = 4, 2048, 1152, 8
HL = 4              # heads per core
HD = 144            # head dim
DV = HL * HD        # 576, v/o width
WQK = 1152          # merged q/k projection width: 512 + 512 + 64 + 64
NT = T // 128       # 16 t-tiles
KC = D // 128       # 9 contraction chunks
SCALE = float(HD) ** -0.5
NCORES = 8

_NC_CACHE = {}


def _build(debug=False):
    import concourse.bacc as bacc
    import concourse.mybir as mybir
    from concourse.tile import TileContext

    dt = mybir.dt
    f32, bf16 = dt.float32, dt.bfloat16
    AF = mybir.ActivationFunctionType

    nc = bacc.Bacc(
        "TRN2",
        target_bir_lowering=False,
        debug=debug,
        enable_asserts=False,
        num_devices=NCORES,
    )

    xT = nc.declare_dram_parameter("xT", [D, T], bf16, isOutput=False)
    wqkT = nc.declare_dram_parameter("wqkT", [D, WQK], bf16, isOutput=False)
    wvT = nc.declare_dram_parameter("wvT", [D, DV], bf16, isOutput=False)
    woT = nc.declare_dram_parameter("woT", [DV, D], bf16, isOutput=False)
    cosN = nc.declare_dram_parameter("cosN", [T, HD], bf16, isOutput=False)
    snPN = nc.declare_dram_parameter("snPN", [T, HD], bf16, isOutput=False)
    identB = nc.declare_dram_parameter("identB", [128, 128], bf16, isOutput=False)
    out = nc.declare_dram_parameter("out", [T, D], f32, isOutput=True)

    with TileContext(nc) as tc:
        with tc.tile_pool(name="persist", bufs=1) as P0:
            ident_bf = P0.tile([128, 128], bf16, name="ident_bf", tag="ident_bf")
            nc.sync.dma_start(ident_bf[:], identB[:])

            qTa = [
                P0.tile([128, T], bf16, name=f"qTa{h}", tag=f"qTa{h}")
                for h in range(HL)
            ]
            kTa = [
                P0.tile([128, T], bf16, name=f"kTa{h}", tag=f"kTa{h}")
                for h in range(HL)
            ]
            qTB = P0.tile([128, T], bf16, name="qTB", tag="qTB")
            kTB = P0.tile([128, T], bf16, name="kTB", tag="kTB")
            # per-head replicas of the b-block rows at all four 32-row groups,
            # so four score-tail K=32 matmuls can issue to distinct PE
            # row-groups and overlap in the array
            qTBr = [
                P0.tile([128, T], bf16, name=f"qTBr{h}", tag=f"qTBr{h}")
                for h in range(HL)
            ]
            kTBr = [
                P0.tile([128, T], bf16, name=f"kTBr{h}", tag=f"kTBr{h}")
                for h in range(HL)
            ]
            vt = [
                P0.tile([128, HL * (HD + 1)], bf16, name=f"v{t}", tag=f"v{t}")
                for t in range(NT)
            ]

            # ---------------- Phase A: projections + rope + transposes -----
            with tc.tile_pool(name="pa", bufs=1) as pa:

                # single SBUF tiles with one DMA descriptor per logical
                # transfer (3D access patterns) — v1/v2 used ~90 descriptors
                # which serialized on the Sync engine (~630 ns each) and
                # gated the first half of the v projection
                xbig = pa.tile([128, KC * T], bf16, name="xbig", tag="xbig")
                x3 = xbig.rearrange("p (c t) -> p c t", c=KC)
                xs = xT.rearrange("(c p) t -> p c t", p=128)
                xtiles = [x3[:, k] for k in range(KC)]
                cos_sb = pa.tile([128, NT * HD], bf16, name="cos_sb", tag="cos_sb")
                snp_sb = pa.tile([128, NT * HD], bf16, name="snp_sb", tag="snp_sb")
                wvbig = pa.tile([128, KC * DV], bf16, name="wvbig", tag="wvbig")
                wv3 = wvbig.rearrange("p (c e) -> p c e", c=KC)
                wv_tiles = [wv3[:, k] for k in range(KC)]
                # first wave at chunk granularity (wv_k + x[k, piece0]
                # interleaved) so matmul (n=0,k) unblocks as pair k lands
                # instead of after the whole wv + piece transfers; later x
                # pieces are single descriptors (Sync-engine issue is
                # ~630 ns per descriptor)
                NP = 8
                PW = T // NP
                wvs = wvT.rearrange("(c p) e -> p c e", p=128)
                for k in range(KC):
                    nc.sync.dma_start(wv3[:, k], wvs[:, k])
                    nc.sync.dma_start(
                        x3[:, k, 0:PW], xs[:, k, 0:PW]
                    )
                for p in range(1, NP):
                    nc.sync.dma_start(
                        x3[:, :, p * PW : (p + 1) * PW],
                        xs[:, :, p * PW : (p + 1) * PW],
                    )

                # ---- v projection ----
                with tc.tile_pool(name="pavps", bufs=1, space="PSUM") as pavps:
                    # dummy matmuls bridge the initial DMA window so HAM
                    # reaches K=8/8 before the first real matmul
                    warm = pavps.tile(
                        [128, 128], f32, name="warm", tag="warm", bufs=1
                    )
                    with tc.high_priority():
                        for _ in range(10):
                            nc.tensor.matmul(
                                warm[:], ident_bf[:], ident_bf[:],
                                start=True, stop=True,
                            )
                    for n in range(NT):
                        ps_v = pavps.tile(
                            [128, DV], f32, name="ps_v", tag="pv", bufs=2
                        )
                        for k in range(KC):
                            st, sp = k == 0, k == KC - 1
                            lhs = xtiles[k][:, n * 128 : (n + 1) * 128]
                            nc.tensor.matmul(
                                ps_v[:, 0:512], lhs, wv_tiles[k][:, 0:512],
                                start=st, stop=sp,
                            )
                            nc.tensor.matmul(
                                ps_v[:, 512:DV], lhs, wv_tiles[k][:, 512:DV],
                                start=st, stop=sp,
                            )
                        v3 = vt[n].rearrange("p (h e) -> p h e", h=HL)
                        nc.vector.tensor_copy(
                            v3[:, :, 0:HD],
                            ps_v.rearrange("p (h e) -> p h e", h=HL),
                        )
                        nc.gpsimd.memset(v3[:, :, HD : HD + 1], 1.0)

                # ---- merged q/k projection weights + trig tables ----
                wqkbig = pa.tile([128, KC * WQK], bf16, name="wqkbig", tag="wqkbig")
                wqk3 = wqkbig.rearrange("p (c e) -> p c e", c=KC)
                wqk_tiles = [wqk3[:, k] for k in range(KC)]
                nc.sync.dma_start(
                    wqk3, wqkT.rearrange("(c p) e -> p c e", p=128)
                )
                nc.sync.dma_start(
                    cos_sb.rearrange("p (n r) -> p n r", n=NT),
                    cosN.rearrange("(n p) r -> p n r", p=128),
                )
                nc.sync.dma_start(
                    snp_sb.rearrange("p (n r) -> p n r", n=NT),
                    snPN.rearrange("(n p) r -> p n r", p=128),
                )

                def trig3(sb, n):
                    # [128, 144] row block for t-tile n, broadcast over 4 heads
                    return (
                        sb[:, n * HD : (n + 1) * HD]
                        .rearrange("p (o r) -> p o r", o=1)
                        .to_broadcast([128, HL, HD])
                    )

                def rope_one(ps_a, ps_bq, qtl, cos3, snp3):
                    """ps_a [128,512] f32 (4 a-blocks), ps_bq [128,4,16] f32
                    view of the packed b dims -> qtl [128,640] bf16 with
                    rotary applied (layout: 4x128 a-blocks | 4x(16+16pad)).

                    m1[j] = q[j]*cos[j]; m2[j] = q[j]*snP[j] where
                    snP[i] = sin[(i+72)%144] * (+1 if i<72 else -1), so every
                    combine is a plain add: out[j] = m1[j] + m2[(j+72)%144].
                    """
                    pa3 = ps_a.rearrange("p (h e) -> p h e", h=HL)
                    m1 = pa.tile([128, 576], bf16, name="m1", tag="m1", bufs=2)
                    m2 = pa.tile([128, 576], bf16, name="m2", tag="m2", bufs=2)
                    m1a = m1[:, 0:512].rearrange("p (h e) -> p h e", h=HL)
                    m1b = m1[:, 512:576].rearrange("p (h e) -> p h e", h=HL)
                    m2a = m2[:, 0:512].rearrange("p (h e) -> p h e", h=HL)
                    m2b = m2[:, 512:576].rearrange("p (h e) -> p h e", h=HL)
                    v = nc.vector
                    v.tensor_mul(m1a[:], pa3[:], cos3[:, :, 0:128])
                    v.tensor_mul(m1b[:], ps_bq[:], cos3[:, :, 128:144])
                    v.tensor_mul(m2a[:], pa3[:], snp3[:, :, 0:128])
                    v.tensor_mul(m2b[:], ps_bq[:], snp3[:, :, 128:144])
                    oa = qtl[:, 0:512].rearrange("p (h e) -> p h e", h=HL)
                    ob = qtl[:, 512:640].rearrange("p (h e) -> p h e", h=HL)
                    # all-bf16 SBUF operands -> DVE 4x fast mode
                    v.tensor_add(oa[:, :, 0:56], m1a[:, :, 0:56], m2a[:, :, 72:128])
                    v.tensor_add(oa[:, :, 56:72], m1a[:, :, 56:72], m2b[:, :, 0:16])
                    v.tensor_add(oa[:, :, 72:128], m1a[:, :, 72:128], m2a[:, :, 0:56])
                    v.tensor_add(ob[:, :, 0:16], m1b[:], m2a[:, :, 56:72])
                    nc.gpsimd.memset(ob[:, :, 16:32], 0.0)

                with tc.tile_pool(name="paqps", bufs=1, space="PSUM") as paqps:

                    def transposes(n, qtl, ktl):
                        for src, dsts, dstb in ((qtl, qTa, qTB), (ktl, kTa, kTB)):
                            for j in range(5):
                                tp = paqps.tile(
                                    [128, 128], bf16, name="tp", tag="tp", bufs=3
                                )
                                nc.tensor.transpose(
                                    tp[:], src[:, 128 * j : 128 * (j + 1)], ident_bf[:]
                                )
                                dst = dsts[j] if j < 4 else dstb
                                nc.scalar.copy(
                                    dst[:, n * 128 : (n + 1) * 128], tp[:]
                                )

                    pend = None
                    for n in range(NT):
                        ps_q = paqps.tile(
                            [128, 512], f32, name="ps_q", tag="psq", bufs=2
                        )
                        ps_k = paqps.tile(
                            [128, 512], f32, name="ps_k", tag="psk", bufs=2
                        )
                        ps_b = paqps.tile(
                            [128, 128], f32, name="ps_b", tag="psb", bufs=1
                        )
                        for k in range(KC):
                            st, sp = k == 0, k == KC - 1
                            lhs = xtiles[k][:, n * 128 : (n + 1) * 128]
                            nc.tensor.matmul(
                                ps_q[:], lhs, wqk_tiles[k][:, 0:512],
                                start=st, stop=sp,
                            )
                            nc.tensor.matmul(
                                ps_k[:], lhs, wqk_tiles[k][:, 512:1024],
                                start=st, stop=sp,
                            )
                            nc.tensor.matmul(
                                ps_b[:], lhs, wqk_tiles[k][:, 1024:1152],
                                start=st, stop=sp,
                            )
                        if pend is not None:
                            transposes(*pend)
                        qtl = pa.tile([128, 640], bf16, name="qtl", tag="qtl", bufs=2)
                        ktl = pa.tile([128, 640], bf16, name="ktl", tag="ktl", bufs=2)
                        c3, s3 = trig3(cos_sb, n), trig3(snp_sb, n)
                        rope_one(
                            ps_q,
                            ps_b[:, 0:64].rearrange("p (h e) -> p h e", h=HL),
                            qtl, c3, s3,
                        )
                        rope_one(
                            ps_k,
                            ps_b[:, 64:128].rearrange("p (h e) -> p h e", h=HL),
                            ktl, c3, s3,
                        )
                        pend = (n, qtl, ktl)
                    transposes(*pend)

                # replicate the b-blocks in two column halves so the
                # first half's DMAs issue as soon as transposes n<=7 are
                # done (subtile deps) instead of after the whole phase;
                # h-major so head 0 lands first
                for half in range(2):
                    cl, cr = half * 1024, (half + 1) * 1024
                    for hh in range(HL):
                        for j in range(4):
                            nc.sync.dma_start(
                                qTBr[hh][32 * j : 32 * j + 32, cl:cr],
                                qTB[32 * hh : 32 * hh + 32, cl:cr],
                            )
                            nc.sync.dma_start(
                                kTBr[hh][32 * j : 32 * j + 32, cl:cr],
                                kTB[32 * hh : 32 * hh + 32, cl:cr],
                            )

            # ---------------- Phase B: attention --------------------------
            with tc.tile_pool(name="pb", bufs=1) as pb:
                ot = [
                    pb.tile([128, DV], bf16, name=f"o{t}", tag=f"o{t}")
                    for t in range(NT)
                ]
                with tc.tile_pool(name="pbps", bufs=1, space="PSUM") as pbps:
                    HD1 = HD + 1
                    NITEM = 2 * HL * NT  # flat (q-pair, head, key-tile) items

                    def decode(idx):
                        p, rem = divmod(idx, HL * NT)
                        h, kt = divmod(rem, NT)
                        return p, h, kt

                    def scores_for(idx):
                        # one key-tile into one 2-bank PSUM tile: per 512-q
                        # half, a K=32 b-block tail first (start=True, two
                        # tails on distinct PE row-groups run concurrently)
                        # then the K=128 main carrying stop; the two mains
                        # share the kTa stationary so its LDWEIGHTS is
                        # hidden. ONE [128,1024] exp.
                        p, h, kt = decode(idx)
                        qof = p * 1024
                        sps = pbps.tile(
                            [128, 1024], f32, name="sps", tag="sc", bufs=2
                        )
                        for half in range(2):
                            rg = 2 * (kt % 2) + half
                            nc.tensor.matmul(
                                sps[:, 512 * half : 512 * (half + 1)],
                                kTBr[h][
                                    32 * rg : 32 * rg + 32,
                                    kt * 128 : (kt + 1) * 128,
                                ],
                                qTBr[h][
                                    32 * rg : 32 * rg + 32,
                                    qof + 512 * half : qof + 512 * (half + 1),
                                ],
                                start=True,
                                stop=False,
                                tile_position=(32 * rg, 0),
                            )
                        for half in range(2):
                            nc.tensor.matmul(
                                sps[:, 512 * half : 512 * (half + 1)],
                                kTa[h][:, kt * 128 : (kt + 1) * 128],
                                qTa[h][
                                    :,
                                    qof + 512 * half : qof + 512 * (half + 1),
                                ],
                                start=False,
                                stop=True,
                            )
                        E = pb.tile(
                            [128, 1024], bf16, name="E", tag="E", bufs=6
                        )
                        nc.scalar.activation(E[:], sps[:], AF.Exp, scale=SCALE)
                        return E

                    # ONE software pipeline (depth 2) across ALL (p,h,kt)
                    # items: scores prefetch crosses (head, q-pair)
                    # boundaries, so the scores->exp->PV chain never
                    # drains/refills between heads (the per-head pipeline
                    # paid an ~1us exp-latency bubble at each of the 8
                    # boundaries)
                    eq = [scores_for(0), scores_for(1)]
                    o_ps = None
                    for idx in range(NITEM):
                        p, h, kt = decode(idx)
                        if kt == 0:
                            # 8 PV accumulators packed into 3 PSUM banks;
                            # O1 double-buffered so this head's first PV
                            # matmuls don't wait on the previous normalize
                            O1 = pbps.tile(
                                [128, 3 * HD1], f32, name="O1", tag="O1", bufs=2
                            )
                            O2 = pbps.tile(
                                [128, 3 * HD1], f32, name="O2", tag="O2", bufs=1
                            )
                            O3 = pbps.tile(
                                [128, 2 * HD1], f32, name="O3", tag="O3", bufs=1
                            )
                            o_ps = (
                                [O1[:, i * HD1 : (i + 1) * HD1] for i in range(3)]
                                + [O2[:, i * HD1 : (i + 1) * HD1] for i in range(3)]
                                + [O3[:, i * HD1 : (i + 1) * HD1] for i in range(2)]
                            )
                        if idx + 2 < NITEM:
                            eq.append(scores_for(idx + 2))
                        E = eq.pop(0)
                        for ql in range(8):
                            # start/stop are carried by the first/last
                            # matmul touching each bank
                            st = kt == 0 and ql in (0, 3, 6)
                            sp = kt == NT - 1 and ql in (2, 5, 7)
                            nc.tensor.matmul(
                                o_ps[ql][:],
                                E[:, ql * 128 : (ql + 1) * 128],
                                vt[kt][:, HD1 * h : HD1 * (h + 1)],
                                start=st,
                                stop=sp,
                            )
                        if kt == NT - 1:
                            last_hp = idx == NITEM - 1
                            # normalize the single-buffered banks (O2: ql
                            # 3-5, O3: ql 6-7) first — the next head's PV
                            # stalls on exactly those; O1 is
                            # double-buffered so its qls can wait
                            for ql in (3, 4, 5, 6, 7, 0, 1, 2):
                                t = 8 * p + ql
                                r = pb.tile(
                                    [128, 1], f32, name="r", tag="r", bufs=4
                                )
                                nc.vector.reciprocal(
                                    r[:], o_ps[ql][:, HD : HD + 1]
                                )
                                dst = ot[t][:, HD * h : HD * (h + 1)]
                                if last_hp and ql % 2 == 1:
                                    # the final normalize chain gates the
                                    # B->C PSUM pool swap; ACT is idle by
                                    # now so split it across two engines
                                    nc.scalar.activation(
                                        dst, o_ps[ql][:, 0:HD], AF.Copy,
                                        scale=r[:],
                                    )
                                else:
                                    nc.vector.tensor_scalar_mul(
                                        dst, o_ps[ql][:, 0:HD], r[:]
                                    )

                # ---------------- Phase C: o^T + final projection ----------
                oTa = [
                    pb.tile([128, T], bf16, name=f"oTa{j}", tag=f"oTa{j}")
                    for j in range(4)
                ]
                oTb = pb.tile([64, T], bf16, name="oTb", tag="oTb")
                wo_tiles = []
                for k in range(5):
                    rows = 128 if k < 4 else 64
                    wot_ = pb.tile([128, D], bf16, name=f"wo{k}", tag=f"wo{k}")
                    nc.sync.dma_start(
                        wot_[0:rows, :], woT[k * 128 : k * 128 + rows, :]
                    )
                    wo_tiles.append(wot_)
                with tc.tile_pool(name="pcps", bufs=1, space="PSUM") as pcps:

                    def o_transp(t):
                        for j in range(4):
                            tp = pcps.tile(
                                [128, 128], bf16, name="tpo", tag="otp", bufs=2
                            )
                            nc.tensor.transpose(
                                tp[:],
                                ot[t][:, 128 * j : 128 * (j + 1)],
                                ident_bf[:],
                            )
                            nc.vector.tensor_copy(
                                oTa[j][:, t * 128 : (t + 1) * 128], tp[:]
                            )
                        tpb = pcps.tile([64, 128], bf16, name="tpb", tag="otp", bufs=2)
                        nc.tensor.transpose(
                            tpb[:],
                            ot[t][:, 512:DV],
                            ident_bf[:],
                        )
                        nc.vector.tensor_copy(
                            oTb[:, t * 128 : (t + 1) * 128], tpb[:]
                        )

                    def final(t):
                        fps = [
                            pcps.tile(
                                [128, 384], f32, name=f"fps{j3}", tag=f"f{j3}", bufs=2
                            )
                            for j3 in range(3)
                        ]
                        # k-outer / j3-inner: the 3 matmuls of each k share
                        # the o^T stationary, hiding its LDWEIGHTS
                        for k in range(5):
                            rows = 128 if k < 4 else 64
                            lhsT = (
                                oTa[k][:, t * 128 : (t + 1) * 128]
                                if k < 4
                                else oTb[:, t * 128 : (t + 1) * 128]
                            )
                            for j3 in range(3):
                                nc.tensor.matmul(
                                    fps[j3][:],
                                    lhsT,
                                    wo_tiles[k][0:rows, 384 * j3 : 384 * (j3 + 1)],
                                    start=(k == 0),
                                    stop=(k == 4),
                                )
                        for j3 in range(3):
                            fout = pb.tile(
                                [128, 384], f32, name="fout", tag="fout", bufs=4
                            )
                            if (t * 3 + j3) % 2 == 1:
                                nc.vector.tensor_copy(fout[:], fps[j3][:])
                            else:
                                nc.scalar.copy(fout[:], fps[j3][:])
                            nc.sync.dma_start(
                                out[
                                    t * 128 : (t + 1) * 128,
                                    384 * j3 : 384 * (j3 + 1),
                                ],
                                fout[:],
                            )

                    o_transp(0)
                    for t in range(NT):
                        if t + 1 < NT:
                            o_transp(t + 1)
                        final(t)

    nc.compile()
    return nc


def get_nc(debug=False):
    key = bool(debug)
    if key not in _NC_CACHE:
        _NC_CACHE[key] = _build(debug)
    return _NC_CACHE[key]


def make_in_maps(x, cos, sin, Wq, Wk, Wv, Wo):
    import ml_dtypes

    x = np.asarray(x, np.float32)
    cos = np.asarray(cos, np.float32)
    sin = np.asarray(sin, np.float32)
    Wq, Wk, Wv, Wo = (np.asarray(w, np.float32) for w in (Wq, Wk, Wv, Wo))
    cos_bf = cos.astype(ml_dtypes.bfloat16)
    # sign-folded, partner-permuted sin: snP[t,i] = sin[t,(i+72)%144] * s,
    # s = +1 for i<72, -1 for i>=72; makes every rope combine a plain add
    snp = sin[:, (np.arange(HD) + 72) % HD].copy()
    snp[:, 72:] *= -1.0
    snp_bf = snp.astype(ml_dtypes.bfloat16)

    in_maps = []
    for c in range(NCORES):
        b, hg = divmod(c, 2)
        heads = [HL * hg + i for i in range(HL)]

        def qk_merged(Wq_, Wk_):
            # rows: [q a-blocks 4x128 | k a-blocks 4x128 | q b 4x16 | k b 4x16]
            Wsel = np.zeros((WQK, D), np.float32)
            for i, g in enumerate(heads):
                Wsel[128 * i : 128 * i + 128] = Wq_[144 * g : 144 * g + 128]
                Wsel[512 + 128 * i : 512 + 128 * i + 128] = Wk_[
                    144 * g : 144 * g + 128
                ]
                Wsel[1024 + 16 * i : 1024 + 16 * i + 16] = Wq_[
                    144 * g + 128 : 144 * g + 144
                ]
                Wsel[1088 + 16 * i : 1088 + 16 * i + 16] = Wk_[
                    144 * g + 128 : 144 * g + 144
                ]
            return np.ascontiguousarray(Wsel.T)

        wv_sel = np.concatenate([Wv[144 * g : 144 * g + 144] for g in heads], 0)
        wo_sel = np.concatenate([Wo[:, 144 * g : 144 * g + 144] for g in heads], 1)
        in_maps.append(
            {
                "xT": np.ascontiguousarray(x[b].T).astype(ml_dtypes.bfloat16),
                "wqkT": qk_merged(Wq, Wk).astype(ml_dtypes.bfloat16),
                "wvT": np.ascontiguousarray(wv_sel.T).astype(ml_dtypes.bfloat16),
                "woT": np.ascontiguousarray(wo_sel.T).astype(ml_dtypes.bfloat16),
                "cosN": cos_bf,
                "snPN": snp_bf,
                "identB": np.eye(128, dtype=ml_dtypes.bfloat16),
            }
        )
    return in_maps


def kernel(x, cos, sin, Wq, Wk, Wv, Wo, _trace=False, _trace_kwargs=None):
    from concourse.bass_utils import run_bass_kernel_spmd

    nc = get_nc()
    in_maps = make_in_maps(x, cos, sin, Wq, Wk, Wv, Wo)
    res = run_bass_kernel_spmd(
        nc,
        in_maps,
        list(range(NCORES)),
        trace=_trace,
        **(_trace_kwargs or {}),
    )
    parts = [res.results[c]["out"] for c in range(NCORES)]
    outb = np.stack([parts[2 * b] + parts[2 * b + 1] for b in range(B)])
    if _trace:
        kernel.last_results = res
    return outb.astype(np.float32)


# revision 25
# speedup vs baseline: 1.1959x; 1.0080x over previous
"""Trainium2 Bass kernel for a fused multi-head attention block.

Reference computation (B=4, T=2048, D=1152, H=8, HD=144, full rotary):
    q,k,v = x@Wq.T, x@Wk.T, x@Wv.T   (per head)
    q,k   = rope(q, k, cos, sin)
    o     = softmax(q k^T / sqrt(HD)) v
    out   = o @ Wo.T

Sharding (8 cores): core c = (batch b = c//2, head-group hg = c%2).
Each core computes 4 heads of one batch and a partial output
out_part = o_local @ Wo[:, hg_cols].T ; host sums the two partials per batch.

Per-core structure (v2 — rebuilt from trace analysis of v1, 476us -> 391us):
  * Projections: q and k are computed in ONE merged pass (wqkT [D,1152] =
    [q a-blocks 512 | k a-blocks 512 | q b-dims 64 | k b-dims 64]) with 3
    matmuls per (n,k-chunk) sharing the x stationary (LDW amortized 1:3).
  * rope reads the projection PSUM directly (no copy): 4 DVE muls using a
    host-precomputed sign-folded/permuted sin table (snP) make every
    combine a plain DVE add; combines write bf16 q/k tiles which PE
    transposes to [e,t].  Transpose results copy out on the Scalar engine
    (idle during phase A).
  * Inputs arrive as ONE DMA descriptor per logical transfer (the Sync
    engine serializes descriptors at ~630 ns — v1's ~90 descriptors gated
    the v projection).  x is piece-major (all 9 D-chunks of each 256-col
    t-piece together) and the first wave is chunk-granular so matmul
    (n=0,k) unblocks as pair k lands.  Dummy warmup matmuls keep HAM at
    K=8/8 through the initial DMA window.  The score-tail b-block replicas
    are DMAed in two column halves so the first half issues mid-phase-A.
  * Attention processes TWO 512-wide q-blocks at once per (head, kt):
    scores S^T [keys, 1024q] into a 2-bank PSUM tile.  Per key-tile: two
    K=32 b-block tails FIRST (start=True, distinct tile_position
    row-groups -> concurrent in PE), then two K=128 mains sharing the kTa
    stationary (LDWEIGHTS hidden), ONE [128,1024] Exp (halves the ACT
    fixed overhead that paced v1), then 8 PV matmuls into 3 PSUM banks of
    packed 145-wide accumulators (softmax denominator via ones column of
    v).  The kt loop is software-pipelined depth 2 (scores run two
    key-tiles ahead of PV) so the PE never idles on scores->exp->PV
    latency; O1 is double-buffered so the next head's PV doesn't wait on
    the normalize.  PSUM budget: 2x2-bank sps + 2+1+1 accumulator banks
    = 8 (the binding constraint throughout).
  * Final projection in bf16: o normalized straight to bf16, PE transpose,
    then k-outer/j3-inner matmuls sharing the o^T stationary 1:3.
  * dtypes: all matmuls bf16 (f32 accum in PSUM); output f32.
"""

import numpy as np

B, T, D, H = 4, 2048, 1152, 8
HL = 4              # heads per core
HD = 144            # head dim
DV = HL * HD        # 576, v/o width
WQK = 1152          # merged q/k projection width: 512 + 512 + 64 + 64
NT = T // 128       # 16 t-tiles
KC = D // 128       # 9 contraction chunks
SCALE = float(HD) ** -0.5
NCORES = 8

_NC_CACHE = {}


def _build(debug=False):
    import concourse.bacc as bacc
    import concourse.mybir as mybir
    from concourse.tile import TileContext

    dt = mybir.dt
    f32, bf16 = dt.float32, dt.bfloat16
    AF = mybir.ActivationFunctionType

    nc = bacc.Bacc(
        "TRN2",
        target_bir_lowering=False,
        debug=debug,
        enable_asserts=False,
        num_devices=NCORES,
    )

    xT = nc.declare_dram_parameter("xT", [D, T], bf16, isOutput=False)
    wqkT = nc.declare_dram_parameter("wqkT", [D, WQK], bf16, isOutput=False)
    wvT = nc.declare_dram_parameter("wvT", [D, DV], bf16, isOutput=False)
    woT = nc.declare_dram_parameter("woT", [DV, D], bf16, isOutput=False)
    cosN = nc.declare_dram_parameter("cosN", [T, HD], bf16, isOutput=False)
    snPN = nc.declare_dram_parameter("snPN", [T, HD], bf16, isOutput=False)
    identB = nc.declare_dram_parameter("identB", [128, 128], bf16, isOutput=False)
    out = nc.declare_dram_parameter("out", [T, D], f32, isOutput=True)

    with TileContext(nc) as tc:
        with tc.tile_pool(name="persist", bufs=1) as P0:
            ident_bf = P0.tile([128, 128], bf16, name="ident_bf", tag="ident_bf")
            nc.sync.dma_start(ident_bf[:], identB[:])

            qTa = [
                P0.tile([128, T], bf16, name=f"qTa{h}", tag=f"qTa{h}")
                for h in range(HL)
            ]
            kTa = [
                P0.tile([128, T], bf16, name=f"kTa{h}", tag=f"kTa{h}")
                for h in range(HL)
            ]
            qTB = P0.tile([128, T], bf16, name="qTB", tag="qTB")
            kTB = P0.tile([128, T], bf16, name="kTB", tag="kTB")
            # per-head replicas of the b-block rows at all four 32-row groups,
            # so four score-tail K=32 matmuls can issue to distinct PE
            # row-groups and overlap in the array
            qTBr = [
                P0.tile([128, T], bf16, name=f"qTBr{h}", tag=f"qTBr{h}")
                for h in range(HL)
            ]
            kTBr = [
                P0.tile([128, T], bf16, name=f"kTBr{h}", tag=f"kTBr{h}")
                for h in range(HL)
            ]
            vt = [
                P0.tile([128, HL * (HD + 1)], bf16, name=f"v{t}", tag=f"v{t}")
                for t in range(NT)
            ]

            # ---------------- Phase A: projections + rope + transposes -----
            with tc.tile_pool(name="pa", bufs=1) as pa:

                # single SBUF tiles with one DMA descriptor per logical
                # transfer (3D access patterns) — v1/v2 used ~90 descriptors
                # which serialized on the Sync engine (~630 ns each) and
                # gated the first half of the v projection
                xbig = pa.tile([128, KC * T], bf16, name="xbig", tag="xbig")
                x3 = xbig.rearrange("p (c t) -> p c t", c=KC)
                xs = xT.rearrange("(c p) t -> p c t", p=128)
                xtiles = [x3[:, k] for k in range(KC)]
                cos_sb = pa.tile([128, NT * HD], bf16, name="cos_sb", tag="cos_sb")
                snp_sb = pa.tile([128, NT * HD], bf16, name="snp_sb", tag="snp_sb")
                wvbig = pa.tile([128, KC * DV], bf16, name="wvbig", tag="wvbig")
                wv3 = wvbig.rearrange("p (c e) -> p c e", c=KC)
                wv_tiles = [wv3[:, k] for k in range(KC)]
                # first wave at chunk granularity (wv_k + x[k, piece0]
                # interleaved) so matmul (n=0,k) unblocks as pair k lands
                # instead of after the whole wv + piece transfers; later x
                # pieces are single descriptors (Sync-engine issue is
                # ~630 ns per descriptor)
                NP = 8
                PW = T // NP
                wvs = wvT.rearrange("(c p) e -> p c e", p=128)
                for k in range(KC):
                    nc.sync.dma_start(wv3[:, k], wvs[:, k])
                    nc.sync.dma_start(
                        x3[:, k, 0:PW], xs[:, k, 0:PW]
                    )
                for p in range(1, NP):
                    nc.sync.dma_start(
                        x3[:, :, p * PW : (p + 1) * PW],
                        xs[:, :, p * PW : (p + 1) * PW],
                    )

                # ---- v projection ----
                with tc.tile_pool(name="pavps", bufs=1, space="PSUM") as pavps:
                    # dummy matmuls bridge the initial DMA window so HAM
                    # reaches K=8/8 before the first real matmul
                    warm = pavps.tile(
                        [128, 128], f32, name="warm", tag="warm", bufs=1
                    )
                    with tc.high_priority():
                        for _ in range(10):
                            nc.tensor.matmul(
                                warm[:], ident_bf[:], ident_bf[:],
                                start=True, stop=True,
                            )
                    for n in range(NT):
                        ps_v = pavps.tile(
                            [128, DV], f32, name="ps_v", tag="pv", bufs=2
                        )
                        for k in range(KC):
                            st, sp = k == 0, k == KC - 1
                            lhs = xtiles[k][:, n * 128 : (n + 1) * 128]
                            nc.tensor.matmul(
                                ps_v[:, 0:512], lhs, wv_tiles[k][:, 0:512],
                                start=st, stop=sp,
                            )
                            nc.tensor.matmul(
                                ps_v[:, 512:DV], lhs, wv_tiles[k][:, 512:DV],
                                start=st, stop=sp,
                            )
                        v3 = vt[n].rearrange("p (h e) -> p h e", h=HL)
                        nc.vector.tensor_copy(
                            v3[:, :, 0:HD],
                            ps_v.rearrange("p (h e) -> p h e", h=HL),
                        )
                        nc.gpsimd.memset(v3[:, :, HD : HD + 1], 1.0)

                # ---- merged q/k projection weights + trig tables ----
                wqkbig = pa.tile([128, KC * WQK], bf16, name="wqkbig", tag="wqkbig")
                wqk3 = wqkbig.rearrange("p (c e) -> p c e", c=KC)
                wqk_tiles = [wqk3[:, k] for k in range(KC)]
                nc.sync.dma_start(
                    wqk3, wqkT.rearrange("(c p) e -> p c e", p=128)
                )
                nc.sync.dma_start(
                    cos_sb.rearrange("p (n r) -> p n r", n=NT),
                    cosN.rearrange("(n p) r -> p n r", p=128),
                )
                nc.sync.dma_start(
                    snp_sb.rearrange("p (n r) -> p n r", n=NT),
                    snPN.rearrange("(n p) r -> p n r", p=128),
                )

                def trig3(sb, n):
                    # [128, 144] row block for t-tile n, broadcast over 4 heads
                    return (
                        sb[:, n * HD : (n + 1) * HD]
                        .rearrange("p (o r) -> p o r", o=1)
                        .to_broadcast([128, HL, HD])
                    )

                def rope_one(ps_a, ps_bq, qtl, cos3, snp3):
                    """ps_a [128,512] f32 (4 a-blocks), ps_bq [128,4,16] f32
                    view of the packed b dims -> qtl [128,640] bf16 with
                    rotary applied (layout: 4x128 a-blocks | 4x(16+16pad)).

                    m1[j] = q[j]*cos[j]; m2[j] = q[j]*snP[j] where
                    snP[i] = sin[(i+72)%144] * (+1 if i<72 else -1), so every
                    combine is a plain add: out[j] = m1[j] + m2[(j+72)%144].
                    """
                    pa3 = ps_a.rearrange("p (h e) -> p h e", h=HL)
                    m1 = pa.tile([128, 576], bf16, name="m1", tag="m1", bufs=2)
                    m2 = pa.tile([128, 576], bf16, name="m2", tag="m2", bufs=2)
                    m1a = m1[:, 0:512].rearrange("p (h e) -> p h e", h=HL)
                    m1b = m1[:, 512:576].rearrange("p (h e) -> p h e", h=HL)
                    m2a = m2[:, 0:512].rearrange("p (h e) -> p h e", h=HL)
                    m2b = m2[:, 512:576].rearrange("p (h e) -> p h e", h=HL)
                    v = nc.vector
                    v.tensor_mul(m1a[:], pa3[:], cos3[:, :, 0:128])
                    v.tensor_mul(m1b[:], ps_bq[:], cos3[:, :, 128:144])
                    v.tensor_mul(m2a[:], pa3[:], snp3[:, :, 0:128])
                    v.tensor_mul(m2b[:], ps_bq[:], snp3[:, :, 128:144])
                    oa = qtl[:, 0:512].rearrange("p (h e) -> p h e", h=HL)
                    ob = qtl[:, 512:640].rearrange("p (h e) -> p h e", h=HL)
                    # all-bf16 SBUF operands -> DVE 4x fast mode
                    v.tensor_add(oa[:, :, 0:56], m1a[:, :, 0:56], m2a[:, :, 72:128])
                    v.tensor_add(oa[:, :, 56:72], m1a[:, :, 56:72], m2b[:, :, 0:16])
                    v.tensor_add(oa[:, :, 72:128], m1a[:, :, 72:128], m2a[:, :, 0:56])
                    v.tensor_add(ob[:, :, 0:16], m1b[:], m2a[:, :, 56:72])
                    nc.gpsimd.memset(ob[:, :, 16:32], 0.0)

                with tc.tile_pool(name="paqps", bufs=1, space="PSUM") as paqps:

                    def transposes(n, qtl, ktl):
                        for src, dsts, dstb in ((qtl, qTa, qTB), (ktl, kTa, kTB)):
                            for j in range(5):
                                tp = paqps.tile(
                                    [128, 128], bf16, name="tp", tag="tp", bufs=3
                                )
                                nc.tensor.transpose(
                                    tp[:], src[:, 128 * j : 128 * (j + 1)], ident_bf[:]
                                )
                                dst = dsts[j] if j < 4 else dstb
                                nc.scalar.copy(
                                    dst[:, n * 128 : (n + 1) * 128], tp[:]
                                )

                    pend = None
                    for n in range(NT):
                        ps_q = paqps.tile(
                            [128, 512], f32, name="ps_q", tag="psq", bufs=2
                        )
                        ps_k = paqps.tile(
                            [128, 512], f32, name="ps_k", tag="psk", bufs=2
                        )
                        ps_b = paqps.tile(
                            [128, 128], f32, name="ps_b", tag="psb", bufs=1
                        )
                        for k in range(KC):
                            st, sp = k == 0, k == KC - 1
                            lhs = xtiles[k][:, n * 128 : (n + 1) * 128]
                            nc.tensor.matmul(
                                ps_q[:], lhs, wqk_tiles[k][:, 0:512],
                                start=st, stop=sp,
                            )
                            nc.tensor.matmul(
                                ps_k[:], lhs, wqk_tiles[k][:, 512:1024],
                                start=st, stop=sp,
                            )
                            nc.tensor.matmul(
                                ps_b[:], lhs, wqk_tiles[k][:, 1024:1152],
                                start=st, stop=sp,
                            )
                        if pend is not None:
                            transposes(*pend)
                        qtl = pa.tile([128, 640], bf16, name="qtl", tag="qtl", bufs=2)
                        ktl = pa.tile([128, 640], bf16, name="ktl", tag="ktl", bufs=2)
                        c3, s3 = trig3(cos_sb, n), trig3(snp_sb, n)
                        rope_one(
                            ps_q,
                            ps_b[:, 0:64].rearrange("p (h e) -> p h e", h=HL),
                            qtl, c3, s3,
                        )
                        rope_one(
                            ps_k,
                            ps_b[:, 64:128].rearrange("p (h e) -> p h e", h=HL),
                            ktl, c3, s3,
                        )
                        pend = (n, qtl, ktl)
                    transposes(*pend)

                # replicate the b-blocks in two column halves so the
                # first half's DMAs issue as soon as transposes n<=7 are
                # done (subtile deps) instead of after the whole phase;
                # h-major so head 0 lands first
                for half in range(2):
                    cl, cr = half * 1024, (half + 1) * 1024
                    for hh in range(HL):
                        for j in range(4):
                            nc.sync.dma_start(
                                qTBr[hh][32 * j : 32 * j + 32, cl:cr],
                                qTB[32 * hh : 32 * hh + 32, cl:cr],
                            )
                            nc.sync.dma_start(
                                kTBr[hh][32 * j : 32 * j + 32, cl:cr],
                                kTB[32 * hh : 32 * hh + 32, cl:cr],
                            )

            # ---------------- Phase B: attention --------------------------
            with tc.tile_pool(name="pb", bufs=1) as pb:
                ot = [
                    pb.tile([128, DV], bf16, name=f"o{t}", tag=f"o{t}")
                    for t in range(NT)
                ]
                with tc.tile_pool(name="pbps", bufs=1, space="PSUM") as pbps:
                    HD1 = HD + 1
                    NITEM = 2 * HL * NT  # flat (q-pair, head, key-tile) items

                    def decode(idx):
                        p, rem = divmod(idx, HL * NT)
                        h, kt = divmod(rem, NT)
                        return p, h, kt

                    def scores_for(idx):
                        # one key-tile into one 2-bank PSUM tile: per 512-q
                        # half, a K=32 b-block tail first (start=True, two
                        # tails on distinct PE row-groups run concurrently)
                        # then the K=128 main carrying stop; the two mains
                        # share the kTa stationary so its LDWEIGHTS is
                        # hidden. ONE [128,1024] exp.
                        p, h, kt = decode(idx)
                        qof = p * 1024
                        sps = pbps.tile(
                            [128, 1024], f32, name="sps", tag="sc", bufs=2
                        )
                        for half in range(2):
                            rg = 2 * (kt % 2) + half
                            nc.tensor.matmul(
                                sps[:, 512 * half : 512 * (half + 1)],
                                kTBr[h][
                                    32 * rg : 32 * rg + 32,
                                    kt * 128 : (kt + 1) * 128,
                                ],
                                qTBr[h][
                                    32 * rg : 32 * rg + 32,
                                    qof + 512 * half : qof + 512 * (half + 1),
                                ],
                                start=True,
                                stop=False,
                                tile_position=(32 * rg, 0),
                            )
                        for half in range(2):
                            nc.tensor.matmul(
                                sps[:, 512 * half : 512 * (half + 1)],
                                kTa[h][:, kt * 128 : (kt + 1) * 128],
                                qTa[h][
                                    :,
                                    qof + 512 * half : qof + 512 * (half + 1),
                                ],
                                start=False,
                                stop=True,
                            )
                        E = pb.tile(
                            [128, 1024], bf16, name="E", tag="E", bufs=6
                        )
                        nc.scalar.activation(E[:], sps[:], AF.Exp, scale=SCALE)
                        return E

                    # ONE software pipeline (depth 2) across ALL (p,h,kt)
                    # items: scores prefetch crosses (head, q-pair)
                    # boundaries, so the scores->exp->PV chain never
                    # drains/refills between heads (the per-head pipeline
                    # paid an ~1us exp-latency bubble at each of the 8
                    # boundaries)
                    eq = [scores_for(0), scores_for(1)]
                    o_ps = None
                    for idx in range(NITEM):
                        p, h, kt = decode(idx)
                        if kt == 0:
                            # 8 PV accumulators packed into 3 PSUM banks;
                            # O1 double-buffered so this head's first PV
                            # matmuls don't wait on the previous normalize
                            O1 = pbps.tile(
                                [128, 3 * HD1], f32, name="O1", tag="O1", bufs=2
                            )
                            O2 = pbps.tile(
                                [128, 3 * HD1], f32, name="O2", tag="O2", bufs=1
                            )
                            O3 = pbps.tile(
                                [128, 2 * HD1], f32, name="O3", tag="O3", bufs=1
                            )
                            o_ps = (
                                [O1[:, i * HD1 : (i + 1) * HD1] for i in range(3)]
                                + [O2[:, i * HD1 : (i + 1) * HD1] for i in range(3)]
                                + [O3[:, i * HD1 : (i + 1) * HD1] for i in range(2)]
                            )
                        if idx + 2 < NITEM:
                            eq.append(scores_for(idx + 2))
                        E = eq.pop(0)
                        for ql in range(8):
                            # start/stop are carried by the first/last
                            # matmul touching each bank
                            st = kt == 0 and ql in (0, 3, 6)
                            sp = kt == NT - 1 and ql in (2, 5, 7)
                            nc.tensor.matmul(
                                o_ps[ql][:],
                                E[:, ql * 128 : (ql + 1) * 128],
                                vt[kt][:, HD1 * h : HD1 * (h + 1)],
                                start=st,
                                stop=sp,
                            )
                        if kt == NT - 1:
                            last_hp = idx == NITEM - 1
                            # normalize the single-buffered banks (O2: ql
                            # 3-5, O3: ql 6-7) first — the next head's PV
                            # stalls on exactly those; O1 is
                            # double-buffered so its qls can wait
                            for ql in (3, 4, 5, 6, 7, 0, 1, 2):
                                t = 8 * p + ql
                                r = pb.tile(
                                    [128, 1], f32, name="r", tag="r", bufs=4
                                )
                                nc.vector.reciprocal(
                                    r[:], o_ps[ql][:, HD : HD + 1]
                                )
                                dst = ot[t][:, HD * h : HD * (h + 1)]
                                if last_hp and ql % 2 == 1:
                                    # the final normalize chain gates the
                                    # B->C PSUM pool swap; ACT is idle by
                                    # now so split it across two engines
                                    nc.scalar.activation(
                                        dst, o_ps[ql][:, 0:HD], AF.Copy,
                                        scale=r[:],
                                    )
                                else:
                                    nc.vector.tensor_scalar_mul(
                                        dst, o_ps[ql][:, 0:HD], r[:]
                                    )

                # ---------------- Phase C: o^T + final projection ----------
                oTa = [
                    pb.tile([128, T], bf16, name=f"oTa{j}", tag=f"oTa{j}")
                    for j in range(4)
                ]
                oTb = pb.tile([64, T], bf16, name="oTb", tag="oTb")
                wo_tiles = []
                for k in range(5):
                    rows = 128 if k < 4 else 64
                    wot_ = pb.tile([128, D], bf16, name=f"wo{k}", tag=f"wo{k}")
                    nc.sync.dma_start(
                        wot_[0:rows, :], woT[k * 128 : k * 128 + rows, :]
                    )
                    wo_tiles.append(wot_)
                with tc.tile_pool(name="pcps", bufs=1, space="PSUM") as pcps:

                    def o_transp(t):
                        for j in range(4):
                            tp = pcps.tile(
                                [128, 128], bf16, name="tpo", tag="otp", bufs=3
                            )
                            nc.tensor.transpose(
                                tp[:],
                                ot[t][:, 128 * j : 128 * (j + 1)],
                                ident_bf[:],
                            )
                            nc.vector.tensor_copy(
                                oTa[j][:, t * 128 : (t + 1) * 128], tp[:]
                            )
                        tpb = pcps.tile([64, 128], bf16, name="tpb", tag="otp", bufs=3)
                        nc.tensor.transpose(
                            tpb[:],
                            ot[t][:, 512:DV],
                            ident_bf[:],
                        )
                        nc.vector.tensor_copy(
                            oTb[:, t * 128 : (t + 1) * 128], tpb[:]
                        )

                    def final(t):
                        fps = [
                            pcps.tile(
                                [128, 384], f32, name=f"fps{j3}", tag=f"f{j3}",
                                bufs=(2 if j3 < 2 else 1),
                            )
                            for j3 in range(3)
                        ]
                        # k-outer / j3-inner: the 3 matmuls of each k share
                        # the o^T stationary, hiding its LDWEIGHTS
                        for k in range(5):
                            rows = 128 if k < 4 else 64
                            lhsT = (
                                oTa[k][:, t * 128 : (t + 1) * 128]
                                if k < 4
                                else oTb[:, t * 128 : (t + 1) * 128]
                            )
                            for j3 in range(3):
                                nc.tensor.matmul(
                                    fps[j3][:],
                                    lhsT,
                                    wo_tiles[k][0:rows, 384 * j3 : 384 * (j3 + 1)],
                                    start=(k == 0),
                                    stop=(k == 4),
                                )
                        for j3 in range(3):
                            fout = pb.tile(
                                [128, 384], f32, name="fout", tag="fout", bufs=4
                            )
                            if (t * 3 + j3) % 2 == 1:
                                nc.vector.tensor_copy(fout[:], fps[j3][:])
                            else:
                                nc.scalar.copy(fout[:], fps[j3][:])
                            nc.sync.dma_start(
                                out[
                                    t * 128 : (t + 1) * 128,
                                    384 * j3 : 384 * (j3 + 1),
                                ],
                                fout[:],
                            )

                    o_transp(0)
                    for t in range(NT):
                        if t + 1 < NT:
                            o_transp(t + 1)
                        final(t)

    nc.compile()
    return nc


def get_nc(debug=False):
    key = bool(debug)
    if key not in _NC_CACHE:
        _NC_CACHE[key] = _build(debug)
    return _NC_CACHE[key]


def make_in_maps(x, cos, sin, Wq, Wk, Wv, Wo):
    import ml_dtypes

    x = np.asarray(x, np.float32)
    cos = np.asarray(cos, np.float32)
    sin = np.asarray(sin, np.float32)
    Wq, Wk, Wv, Wo = (np.asarray(w, np.float32) for w in (Wq, Wk, Wv, Wo))
    cos_bf = cos.astype(ml_dtypes.bfloat16)
    # sign-folded, partner-permuted sin: snP[t,i] = sin[t,(i+72)%144] * s,
    # s = +1 for i<72, -1 for i>=72; makes every rope combine a plain add
    snp = sin[:, (np.arange(HD) + 72) % HD].copy()
    snp[:, 72:] *= -1.0
    snp_bf = snp.astype(ml_dtypes.bfloat16)

    in_maps = []
    for c in range(NCORES):
        b, hg = divmod(c, 2)
        heads = [HL * hg + i for i in range(HL)]

        def qk_merged(Wq_, Wk_):
            # rows: [q a-blocks 4x128 | k a-blocks 4x128 | q b 4x16 | k b 4x16]
            Wsel = np.zeros((WQK, D), np.float32)
            for i, g in enumerate(heads):
                Wsel[128 * i : 128 * i + 128] = Wq_[144 * g : 144 * g + 128]
                Wsel[512 + 128 * i : 512 + 128 * i + 128] = Wk_[
                    144 * g : 144 * g + 128
                ]
                Wsel[1024 + 16 * i : 1024 + 16 * i + 16] = Wq_[
                    144 * g + 128 : 144 * g + 144
                ]
                Wsel[1088 + 16 * i : 1088 + 16 * i + 16] = Wk_[
                    144 * g + 128 : 144 * g + 144
                ]
            return np.ascontiguousarray(Wsel.T)

        wv_sel = np.concatenate([Wv[144 * g : 144 * g + 144] for g in heads], 0)
        wo_sel = np.concatenate([Wo[:, 144 * g : 144 * g + 144] for g in heads], 1)
        in_maps.append(
            {
                "xT": np.ascontiguousarray(x[b].T).astype(ml_dtypes.bfloat16),
                "wqkT": qk_merged(Wq, Wk).astype(ml_dtypes.bfloat16),
                "wvT": np.ascontiguousarray(wv_sel.T).astype(ml_dtypes.bfloat16),
                "woT": np.ascontiguousarray(wo_sel.T).astype(ml_dtypes.bfloat16),
                "cosN": cos_bf,
                "snPN": snp_bf,
                "identB": np.eye(128, dtype=ml_dtypes.bfloat16),
            }
        )
    return in_maps


def kernel(x, cos, sin, Wq, Wk, Wv, Wo, _trace=False, _trace_kwargs=None):
    from concourse.bass_utils import run_bass_kernel_spmd

    nc = get_nc()
    in_maps = make_in_maps(x, cos, sin, Wq, Wk, Wv, Wo)
    res = run_bass_kernel_spmd(
        nc,
        in_maps,
        list(range(NCORES)),
        trace=_trace,
        **(_trace_kwargs or {}),
    )
    parts = [res.results[c]["out"] for c in range(NCORES)]
    outb = np.stack([parts[2 * b] + parts[2 * b + 1] for b in range(B)])
    if _trace:
        kernel.last_results = res
    return outb.astype(np.float32)
